# revision 61
# baseline (speedup 1.0000x reference)
"""Trainium2 Bass kernel for nn_AdaptiveFunctionBlock (gnn_message_passing).

Row-shards N=4096 across 8 NeuronCores (512 rows each).  Host prepacks
bf16 / pre-transposed views of the inputs (X, X^T, S_loc^T, [W3|W1@W2],
U_s) so the device does no layout shuffles or casts of the raw inputs.
S^T and A^T live in SBUF for the whole kernel (no HBM scratch).

Per core:
  prologue: rowsums of S (PE ones-matmul), KP^T = [W3|W1W2]^T @ X^T
    (full, so no K/P AllGather), K1P1 row-major via PE transposes.
  step s: L = P@K^T (bf16 matmuls, f32 psum), exact per-row top-p via
    segment top-8 candidates -> sorted top-T -> f32 cumsum threshold
    (reference-faithful tie handling on step 2), softmax -> A (bf16),
    A^T via PE transposes into resident SBUF, then row-parallel GEMMs
    Xa' = A@Xa, Xf' = recipD*(S@Xf + Xf_diag), Z += Xf'@U1_s + Xa'@U2_s,
    with an AllGather of (Xf'|Xa') between steps and a small AllGather
    of K2P2 for the step-2 logits.  Finally LayerNorm(X+Z)*gamma+beta.
"""

import numpy as np

N = 4096
D = 1024
DA = 64
NCORES = 8
NLOC = N // NCORES          # 512
NT = NLOC // 128            # 4 row-tiles per core
NJC = N // 128              # 32 column chunks
P_TOP = 0.9
LN_EPS = 1e-5
T1, T2 = 16, 96             # top-p extraction depth per step
NSEG = 32
SEGW = N // NSEG            # 128
NEGINF = -3.0e38
POSINF = 3.0e38

_CACHE = {}


def _build(ln_id=False):
    import concourse.bass as bass
    import concourse.mybir as mybir
    from concourse import bacc, tile

    dt = mybir.dt
    f32 = dt.float32
    f32r = dt.float32r
    bf16 = dt.bfloat16
    f8 = dt.float8e4
    DR = mybir.MatmulPerfMode.DoubleRow
    Act = mybir.ActivationFunctionType
    Alu = mybir.AluOpType
    PSUM = bass.MemorySpace.PSUM
    DRAM = bass.MemorySpace.DRAM

    nc = bacc.Bacc(num_devices=NCORES)

    # ---------------- I/O ----------------
    f16 = dt.float16
    Xb_d = nc.dram_tensor("Xb", [N, D], bf16, kind="ExternalInput")
    Xb8_d = nc.dram_tensor("Xb8", [N, D], f8, kind="ExternalInput")
    Xloc_d = nc.dram_tensor("Xloc", [NLOC, D], f32, kind="ExternalInput")
    XlocThi_d = nc.dram_tensor("XlocThi", [128, 8, NLOC], f16, kind="ExternalInput")
    XlocTlo_d = nc.dram_tensor("XlocTlo", [128, 8, NLOC], f16, kind="ExternalInput")
    STb_d = nc.dram_tensor("STb", [128, NJC, NLOC], f8, kind="ExternalInput")
    W31h_d = nc.dram_tensor("W31h", [128, 8, 128], f16, kind="ExternalInput")
    W31l_d = nc.dram_tensor("W31l", [128, 8, 128], f16, kind="ExternalInput")
    U_d = [
        nc.dram_tensor("U1_0", [D, D], bf16, kind="ExternalInput"),
        nc.dram_tensor("U2_0", [D, D], bf16, kind="ExternalInput"),
        nc.dram_tensor("U1_1", [D, D], bf16, kind="ExternalInput"),
        nc.dram_tensor("U2_1", [D, D], bf16, kind="ExternalInput"),
    ]
    gamma_d = nc.dram_tensor("gamma", [1, D], f32, kind="ExternalInput")
    beta_d = nc.dram_tensor("beta", [1, D], f32, kind="ExternalInput")
    out_d = nc.dram_tensor("out", [NLOC, D], f32, kind="ExternalOutput")

    # ---------------- inline constants ----------------
    identb_c = nc.inline_tensor(
        np.eye(128, dtype=np.float32), name="identb_c"
    )
    ones1_c = nc.inline_tensor(np.ones((1, 128), dtype=np.float32), name="ones1_c")
    onesp_c = nc.inline_tensor(np.ones((128, 1), dtype=np.float32), name="onesp_c")

    rg = [list(range(NCORES))]
    Tmax = max(T1, T2)

    with tile.TileContext(nc) as tc:
        with (
            tc.tile_pool(name="const", bufs=1) as cpool,
            tc.tile_pool(name="res", bufs=1) as rpool,
            tc.tile_pool(name="big", bufs=1) as bpool,
            tc.tile_pool(name="stream", bufs=1) as stpool,
            tc.tile_pool(name="small", bufs=1) as spool,
            tc.tile_pool(name="psum", bufs=1, space=PSUM) as ppool,
            tc.tile_pool(name="dram", bufs=1, space=DRAM) as dpool,
        ):
            def ps_tile(name):
                return ppool.tile([128, 512], f32, tag="pb", bufs=8, name=name)

            def ps_tile_b(name):
                return ppool.tile([128, 512], bf16, tag="pb", bufs=8, name=name)

            # ---------- SBUF consts ----------
            identf_s = cpool.tile([128, 128], f32, name="identf_s")
            nc.sync.dma_start(identf_s[:], identb_c[:])
            identr_s = cpool.tile([128, 128], f32r, name="identr_s")
            nc.sync.dma_start(identr_s[:], identb_c[:].bitcast(f32r))
            identb_s = cpool.tile([128, 128], bf16, name="identb_s")
            nc.vector.tensor_copy(identb_s[:], identf_s[:])
            ones1_s = cpool.tile([1, 128], f32, name="ones1_s")
            nc.sync.dma_start(ones1_s[:], ones1_c[:])
            onespf_s = cpool.tile([128, 1], f32, name="onespf_s")
            nc.sync.dma_start(onespf_s[:], onesp_c[:])
            onespb_s = cpool.tile([128, 1], f8, name="onespb_s")
            nc.vector.tensor_copy(onespb_s[:], onespf_s[:])
            zerosT_s = cpool.tile([128, Tmax], f32, name="zerosT_s")
            nc.vector.memset(zerosT_s[:], 0.0)

            # ---------- collective DRAM buffers ----------
            # merged AllGather payload: Xf' (NLOC x D f8 viewed as f32) then
            # kp2 (64 x NLOC f32 flattened into 128 rows of 256)
            ag_m_in = dpool.tile([NLOC + 128, 256], f32, name="ag_m_in")
            ag_m_out = dpool.tile(
                [NCORES, NLOC + 128, 256], f32, addr_space="Shared", name="ag_m_out"
            )
            # prologue KP^T AllGather: each core contributes its local cols
            ag_kp1_in = dpool.tile([128, NLOC], f32, name="ag_kp1_in")
            ag_kp1_out = dpool.tile(
                [NCORES, 128, NLOC], f32, addr_space="Shared", name="ag_kp1_out"
            )
            ag_xa_in = dpool.tile([NLOC, D], bf16, name="ag_xa_in")
            ag_xa_out = dpool.tile(
                [NCORES, NLOC, D], bf16, addr_space="Shared", name="ag_xa_out"
            )

            # ---------- residents ----------
            recipD_s = rpool.tile([128, NT], f32, name="recipD_s")
            Z_s = rpool.tile([128, NT, D], f32, name="Z_s")
            ST_s = rpool.tile([128, NJC, NLOC], f8, name="ST_s")
            ATb_s = rpool.tile([128, NT, N], bf16, name="ATb_s")
            K1P1rm_s = rpool.tile([128, NJC * 128], bf16, tag="K1P1", bufs=1, name="K1P1rm_s")
            Xf_loc = rpool.tile([128, NT, D], bf16, name="Xf_loc")
            Xa_loc = rpool.tile([128, NT, D], bf16, name="Xa_loc")
            KP2_s = rpool.tile([128, NLOC], f32r, name="KP2_s")

            # ================= PROLOGUE =================
            # S^T resident (prepacked on host, fully contiguous load)
            nc.sync.dma_start(ST_s[:], STb_d[:])
            W31h_s = stpool.tile([128, 8, 128], f16, tag="x4k", bufs=2, name="W31h_s")
            nc.sync.dma_start(W31h_s[:], W31h_d[:])
            W31l_s = stpool.tile([128, 8, 128], f16, tag="x4k", bufs=2, name="W31l_s")
            nc.sync.dma_start(W31l_s[:], W31l_d[:])

            # local KP^T = W31^T @ Xloc^T  (fp16 hi/lo 3-pass: ~exact), then
            # AllGather the 512-col slices so every core gets full KP^T.
            xlh = bpool.tile([128, 8, NLOC], f16, tag="L", bufs=2, name="xlh")
            nc.sync.dma_start(xlh[:], XlocThi_d[:])
            xll = bpool.tile([128, 8, NLOC], f16, tag="L", bufs=2, name="xll")
            nc.sync.dma_start(xll[:], XlocTlo_d[:])
            kqlp = ps_tile("kqlp")
            for dc in range(8):
                nc.tensor.matmul(
                    kqlp[:], W31h_s[:, dc, :], xlh[:, dc, :],
                    start=(dc == 0), stop=False,
                )
                nc.tensor.matmul(
                    kqlp[:], W31l_s[:, dc, :], xlh[:, dc, :], start=False, stop=False
                )
                nc.tensor.matmul(
                    kqlp[:], W31h_s[:, dc, :], xll[:, dc, :],
                    start=False, stop=(dc == 7),
                )
            KPloc_s = spool.tile([128, NLOC], f32, tag="PT", bufs=2, name="KPloc_s")
            nc.scalar.activation(KPloc_s[:], kqlp[:], Act.Copy)
            PT1_s = spool.tile([128, NLOC], f32, tag="PT", bufs=2, name="PT1_s")
            nc.sync.dma_start(PT1_s[0:64, :], KPloc_s[64:128, :])
            nc.sync.dma_start(PT1_s[64:128, :], KPloc_s[64:128, :])
            nc.sync.dma_start(ag_kp1_in[:], KPloc_s[:])
            nc.gpsimd.collective_compute(
                "AllGather",
                Alu.bypass,
                replica_groups=rg,
                ins=[ag_kp1_in[:].opt()],
                outs=[ag_kp1_out[:].opt()],
            )

            # assemble K1 two-half layout + full KP^T from the gather
            KPT_s = bpool.tile([128, N], f32, tag="E", bufs=1, name="KPT_s")
            K1Th_s = rpool.tile([128, N // 2], f32, tag="KTh", bufs=1, name="K1Th_s")
            for co in range(8):
                nc.sync.dma_start(
                    KPT_s[:, co * 512 : (co + 1) * 512], ag_kp1_out[co, :, :]
                )
                ph = 0 if co < 4 else 64
                off = co if co < 4 else co - 4
                nc.sync.dma_start(
                    K1Th_s[ph : ph + 64, off * 512 : (off + 1) * 512],
                    ag_kp1_out[co, 0:64, :],
                )

            # K1P1 row-major bf16 (for K2P2 accumulation) via f32 PE transposes
            for jq in range(8):
                tpk = ps_tile(f"tpk{jq}")
                for b in range(4):
                    jc = jq * 4 + b
                    nc.tensor.transpose(
                        tpk[:, b * 128 : (b + 1) * 128],
                        KPT_s[:, jc * 128 : (jc + 1) * 128],
                        identf_s[:],
                    )
                nc.scalar.activation(
                    K1P1rm_s[:, jq * 512 : (jq + 1) * 512], tpk[:], Act.Copy
                )

            kp2ps = None
            K2Th_s = None
            PT2_s = None

            # ================= STEPS =================
            for s in range(2):
                T = T1 if s == 0 else T2
                if s == 0:
                    PTl, KTh = PT1_s, K1Th_s
                else:
                    PTl, KTh = PT2_s, K2Th_s
                Ab_t = [None] * NT

                def emit_post(it, s=s):
                    # A^T into resident SBUF (bf16 PE transposes), then the
                    # step-0 K2P2 accumulation for this row tile.
                    Ab = Ab_t[it]
                    for jq in range(8):
                        tpa = ps_tile_b(f"tpa{s}{it}{jq}")
                        for b in range(4):
                            jc = jq * 4 + b
                            nc.tensor.transpose(
                                tpa[:, b * 128 : (b + 1) * 128],
                                Ab[:, jc * 128 : (jc + 1) * 128],
                                identb_s[:],
                            )
                        nc.scalar.activation(
                            ATb_s[:, it, jq * 512 : (jq + 1) * 512], tpa[:], Act.Copy
                        )
                    if s == 0:
                        for jc in range(NJC):
                            nc.tensor.matmul(
                                kp2ps[:, it * 128 : (it + 1) * 128],
                                K1P1rm_s[:, jc * 128 : (jc + 1) * 128],
                                ATb_s[:, it, jc * 128 : (jc + 1) * 128],
                                start=(jc == 0),
                                stop=(jc == NJC - 1),
                            )

                # xf-part: pair-split passes, 2 psum banks live at a time
                xfp = {}

                def emit_xf_pass(pr, dh, s=s):
                    xfp[(pr, dh)] = [
                        ps_tile(f"xfp{s}{pr}{dh}a"),
                        ps_tile(f"xfp{s}{pr}{dh}b"),
                    ]
                    for q2 in range(NJC // 2):
                        xch = stpool.tile(
                            [128, 2, 512], f8, tag="xch5", bufs=3, name="xch"
                        )
                        for kt in range(2):
                            jc = 2 * q2 + kt
                            if s == 0:
                                nc.sync.dma_start(
                                    xch[:, kt, :],
                                    Xb8_d[
                                        jc * 128 : (jc + 1) * 128,
                                        dh * 512 : (dh + 1) * 512,
                                    ],
                                )
                            else:
                                r_, blk_ = jc // NT, jc % NT
                                nc.sync.dma_start(
                                    xch[:, kt, :],
                                    ag_m_out[
                                        r_,
                                        blk_ * 128 : (blk_ + 1) * 128,
                                        dh * 128 : (dh + 1) * 128,
                                    ].bitcast(f8),
                                )
                        for j in range(2):
                            itt = 2 * pr + j
                            nc.tensor.matmul(
                                xfp[(pr, dh)][j][:],
                                ST_s[:, 2 * q2 : 2 * q2 + 2, itt * 128 : (itt + 1) * 128],
                                xch[:],
                                start=(q2 == 0),
                                stop=(q2 == NJC // 2 - 1),
                                perf_mode=DR,
                            )
                    # Xf' = recipD*(psum + xdiag); frees the pass banks
                    for j in range(2):
                        it = 2 * pr + j
                        if s == 0:
                            xdg = stpool.tile(
                                [128, 512], f32, tag="xth", bufs=2, name="xdg"
                            )
                            nc.sync.dma_start(
                                xdg[:],
                                Xloc_d[
                                    it * 128 : (it + 1) * 128,
                                    dh * 512 : (dh + 1) * 512,
                                ],
                            )
                            src = xdg[:]
                        else:
                            src = Xf_loc[:, it, dh * 512 : (dh + 1) * 512]
                        tsum = stpool.tile(
                            [128, 512], f32, tag="xth", bufs=2, name="tsum"
                        )
                        nc.vector.tensor_tensor(
                            tsum[:], xfp[(pr, dh)][j][:], src, Alu.add
                        )
                        nc.vector.tensor_scalar_mul(
                            Xf_loc[:, it, dh * 512 : (dh + 1) * 512],
                            tsum[:],
                            recipD_s[:, it : it + 1],
                        )

                def emit_xa_pass(pr, dh, s=s):
                    xab = [ps_tile(f"xap{s}{pr}{dh}a"), ps_tile(f"xap{s}{pr}{dh}b")]
                    for jc in range(NJC):
                        xch = stpool.tile(
                            [128, 512], bf16, tag="xchw", bufs=3, name="xcha"
                        )
                        if s == 0:
                            nc.sync.dma_start(
                                xch[:],
                                Xb_d[
                                    jc * 128 : (jc + 1) * 128,
                                    dh * 512 : (dh + 1) * 512,
                                ],
                            )
                        else:
                            r_, blk_ = jc // NT, jc % NT
                            nc.sync.dma_start(
                                xch[:],
                                ag_xa_out[
                                    r_,
                                    blk_ * 128 : (blk_ + 1) * 128,
                                    dh * 512 : (dh + 1) * 512,
                                ],
                            )
                        for j in range(2):
                            it = 2 * pr + j
                            nc.tensor.matmul(
                                xab[j][:],
                                ATb_s[:, it, jc * 128 : (jc + 1) * 128],
                                xch[:],
                                start=(jc == 0),
                                stop=(jc == NJC - 1),
                            )
                    for j in range(2):
                        it = 2 * pr + j
                        nc.scalar.activation(
                            Xa_loc[:, it, dh * 512 : (dh + 1) * 512],
                            xab[j][:],
                            Act.Copy,
                        )

                def emit_tpt(pr, s=s):
                    # Xf^T / Xa^T for the pair's tiles
                    for src, dst in ((Xf_loc, XfT_s), (Xa_loc, XaT_s)):
                        for dc in range(8):
                            tpt = ps_tile_b(f"tpt{s}{pr}{dc}")
                            for j in range(2):
                                it = 2 * pr + j
                                nc.tensor.transpose(
                                    tpt[:, j * 128 : (j + 1) * 128],
                                    src[:, it, dc * 128 : (dc + 1) * 128],
                                    identb_s[:],
                                )
                            nc.scalar.activation(
                                dst[:, dc, pr * 256 : (pr + 1) * 256], tpt[:, 0:256],
                                Act.Copy,
                            )

                def emit_u_pass(pr, s=s):
                    # Z(pair) = Xf'@U1_s + Xa'@U2_s (+ X on step 0)
                    zpb = [[ps_tile(f"zp{s}{pr}{j}{dh}") for dh in range(2)]
                           for j in range(2)]
                    for u, XT in ((0, XfT_s), (1, XaT_s)):
                        for dc in range(8):
                            ubf = stpool.tile(
                                [128, D], bf16, tag="x2k", bufs=2, name="ubf"
                            )
                            nc.sync.dma_start(
                                ubf[:], U_d[2 * s + u][dc * 128 : (dc + 1) * 128, :]
                            )
                            for j in range(2):
                                it = 2 * pr + j
                                for dh in range(2):
                                    nc.tensor.matmul(
                                        zpb[j][dh][:],
                                        XT[:, dc, it * 128 : (it + 1) * 128],
                                        ubf[:, dh * 512 : (dh + 1) * 512],
                                        start=(u == 0 and dc == 0),
                                        stop=(u == 1 and dc == 7),
                                    )
                    for j in range(2):
                        it = 2 * pr + j
                        for dh in range(2):
                            if s == 0:
                                xlf = stpool.tile(
                                    [128, 512], f32, tag="xth", bufs=2, name="xlf0"
                                )
                                nc.sync.dma_start(
                                    xlf[:],
                                    Xloc_d[
                                        it * 128 : (it + 1) * 128,
                                        dh * 512 : (dh + 1) * 512,
                                    ],
                                )
                                nc.vector.tensor_tensor(
                                    Z_s[:, it, dh * 512 : (dh + 1) * 512],
                                    zpb[j][dh][:],
                                    xlf[:],
                                    Alu.add,
                                )
                            else:
                                nc.vector.tensor_tensor(
                                    Z_s[:, it, dh * 512 : (dh + 1) * 512],
                                    Z_s[:, it, dh * 512 : (dh + 1) * 512],
                                    zpb[j][dh][:],
                                    Alu.add,
                                )

                def emit_lp(it, s=s):
                    L = bpool.tile([128, N], f32, tag="L", bufs=2, name="L")
                    for jc8 in range(8):
                        lp = ps_tile(f"lp{it}{jc8}")
                        kph = 0 if jc8 < 4 else 64
                        kpo = jc8 if jc8 < 4 else jc8 - 4
                        nc.tensor.matmul(
                            lp[:],
                            PTl[kph : kph + 64, it * 128 : (it + 1) * 128],
                            KTh[kph : kph + 64, kpo * 512 : (kpo + 1) * 512],
                            start=True,
                            stop=True,
                        )
                        nc.scalar.activation(
                            L[:, jc8 * 512 : (jc8 + 1) * 512], lp[:], Act.Copy
                        )
                    return L

                if s == 0:
                    # rowsums of S (+1 for the identity) -> recipD
                    kp2ps = ps_tile("kp2ps")
                    for rit in range(NT):
                        rsp = ps_tile(f"rsp{rit}")
                        for jc in range(NJC):
                            nc.tensor.matmul(
                                rsp[:, 0:1],
                                ST_s[:, jc, rit * 128 : (rit + 1) * 128],
                                onespb_s[:],
                                start=(jc == 0),
                                stop=(jc == NJC - 1),
                            )
                        rsf = spool.tile([128, 1], f32, tag="rs", bufs=2, name="rsf")
                        nc.vector.tensor_scalar_add(rsf[:], rsp[:, 0:1], 1.0)
                        nc.vector.reciprocal(recipD_s[:, rit : rit + 1], rsf[:])

                L_t = [None] * NT
                L_t[0] = emit_lp(0)
                L_t[1] = emit_lp(1)
                emit_xf_pass(0, 0)
                emit_xf_pass(0, 1)
                emit_xf_pass(1, 0)
                emit_xf_pass(1, 1)

                def emit_topp(it, s=s, T=T):
                    L = L_t[it]
                    # --- candidates: per-segment top-8
                    cand = spool.tile([128, 256], f32, tag="cand", bufs=2, name="cand")
                    for sg in range(NSEG):
                        nc.vector.max(
                            cand[:, sg * 8 : sg * 8 + 8],
                            L[:, sg * SEGW : (sg + 1) * SEGW],
                        )
                    if s == 1:
                        cand_copy = spool.tile(
                            [128, 256], f32, tag="cand", bufs=2, name="cand_copy"
                        )
                        nc.vector.tensor_copy(cand_copy[:], cand[:])

                    # --- extract sorted top-T (destroys cand)
                    V = spool.tile([128, Tmax], f32, tag="V", bufs=2, name="V")
                    for rnd in range(T // 8):
                        nc.vector.max(V[:, rnd * 8 : rnd * 8 + 8], cand[:])
                        if rnd < T // 8 - 1:
                            nc.vector.match_replace(
                                cand[:], V[:, rnd * 8 : rnd * 8 + 8], cand[:], NEGINF
                            )

                    negm = spool.tile([128, 1], f32, tag="negm", bufs=2, name="negm")
                    nc.vector.tensor_scalar_mul(negm[:], V[:, 0:1], -1.0)

                    E = bpool.tile([128, N], f32, tag="E", bufs=1, name="E")
                    Zrow = spool.tile([128, 1], f32, tag="Zrow", bufs=2, name="Zrow")
                    nc.scalar.activation(
                        E[:], L[:], Act.Exp, bias=negm[:], accum_out=Zrow[:]
                    )
                    EV = spool.tile([128, Tmax], f32, tag="EV", bufs=2, name="EV")
                    nc.scalar.activation(EV[:, 0:T], V[:, 0:T], Act.Exp, bias=negm[:])
                    cs = spool.tile([128, Tmax], f32, tag="cs", bufs=2, name="cs")
                    nc.vector.tensor_tensor_scan(
                        cs[:, 0:T], EV[:, 0:T], zerosT_s[:, 0:T], 0.0, Alu.add, Alu.add
                    )
                    thr = spool.tile([128, 1], f32, tag="thr", bufs=2, name="thr")
                    nc.vector.tensor_scalar_mul(thr[:], Zrow[:], P_TOP)
                    kept = spool.tile([128, Tmax], f32, tag="kept", bufs=2, name="kept")
                    nc.vector.scalar_tensor_tensor(
                        kept[:, 0:T], cs[:, 0:T], thr[:], EV[:, 0:T],
                        Alu.subtract, Alu.is_lt,
                    )
                    scr1 = spool.tile([128, Tmax], f32, tag="scr1", bufs=1, name="scr1")
                    Drow = spool.tile([128, 1], f32, tag="Drow", bufs=2, name="Drow")
                    nc.vector.tensor_tensor(
                        scr1[:, 0:T], EV[:, 0:T], kept[:, 0:T], Alu.mult
                    )
                    nc.vector.tensor_reduce(
                        Drow[:], scr1[:, 0:T], mybir.AxisListType.X, Alu.add
                    )
                    nki = spool.tile([128, Tmax], f32, tag="nki", bufs=1, name="nki")
                    nc.vector.tensor_scalar(
                        nki[:, 0:T], kept[:, 0:T], 0.5, POSINF, Alu.is_lt, Alu.mult
                    )
                    scr2 = spool.tile([128, Tmax], f32, tag="scr2", bufs=1, name="scr2")
                    tau = spool.tile([128, 1], f32, tag="tau", bufs=2, name="tau")
                    nc.vector.tensor_tensor(
                        scr2[:, 0:T], nki[:, 0:T], V[:, 0:T], Alu.add
                    )
                    nc.vector.tensor_reduce(
                        tau[:], scr2[:, 0:T], mybir.AxisListType.X, Alu.min
                    )
                    recD = spool.tile([128, 1], f32, tag="recD", bufs=2, name="recD")
                    nc.vector.reciprocal(recD[:], Drow[:])
                    etau = spool.tile([128, 1], f32, tag="etau", bufs=2, name="etau")
                    nc.scalar.activation(etau[:], tau[:], Act.Exp, bias=negm[:])

                    if s == 1:
                        scrT = spool.tile(
                            [128, Tmax], f32, tag="scrT", bufs=1, name="scrT"
                        )
                        rr = spool.tile([128, 1], f32, tag="rr", bufs=2, name="rr")
                        nc.vector.scalar_tensor_tensor(
                            scrT[:, 0:T], V[:, 0:T], tau[:], kept[:, 0:T],
                            Alu.is_equal, Alu.mult,
                        )
                        nc.vector.tensor_reduce(
                            rr[:], scrT[:, 0:T], mybir.AxisListType.X, Alu.add
                        )
                        scr256 = spool.tile(
                            [128, 256], f32, tag="scrT", bufs=1, name="scr256"
                        )
                        ceq = spool.tile([128, 1], f32, tag="ceq", bufs=2, name="ceq")
                        nc.vector.tensor_scalar(
                            scr256[:], cand_copy[:], tau[:], None, Alu.is_equal
                        )
                        nc.vector.tensor_reduce(
                            ceq[:], scr256[:], mybir.AxisListType.X, Alu.add
                        )
                        # w2 = (ceq - r)/ceq * etau * recD  (spread over all ties)
                        wv = spool.tile([128, 1], f32, tag="wv", bufs=2, name="wv")
                        nc.vector.tensor_tensor(wv[:], ceq[:], rr[:], Alu.subtract)
                        nc.vector.tensor_tensor(wv[:], wv[:], etau[:], Alu.mult)
                        nc.vector.tensor_tensor(wv[:], wv[:], recD[:], Alu.mult)
                        rceq = spool.tile([128, 1], f32, tag="rceq", bufs=2, name="rceq")
                        nc.vector.reciprocal(rceq[:], ceq[:])
                        nc.vector.tensor_tensor(wv[:], wv[:], rceq[:], Alu.mult)

                    # --- A materialization: E := (E >= etau) * E;  Ab = E*recD
                    # (step 2 fuses the tie-drop correction into the scale)
                    nc.vector.scalar_tensor_tensor(
                        E[:], E[:], etau[:], E[:], Alu.is_ge, Alu.mult
                    )
                    Ab = bpool.tile([128, N], bf16, tag="XfT", bufs=2, name="Ab")
                    if s == 1:
                        eqw = rpool.tile([128, N], bf16, tag="K1P1", bufs=1, name="eqw")
                        nc.vector.tensor_scalar(
                            eqw[:], L[:], tau[:], wv[:], Alu.is_equal, Alu.mult
                        )
                        nc.vector.scalar_tensor_tensor(
                            Ab[:], E[:], recD[:], eqw[:], Alu.mult, Alu.subtract
                        )
                    else:
                        nc.scalar.activation(Ab[:], E[:], Act.Copy, scale=recD[:])
                    Ab_t[it] = Ab

                emit_topp(0)
                emit_topp(1)
                L_t[2] = emit_lp(2)
                L_t[3] = emit_lp(3)
                emit_post(0)
                emit_post(1)
                emit_xa_pass(0, 0)
                emit_xa_pass(0, 1)
                emit_topp(2)
                emit_topp(3)
                XfT_s = bpool.tile(
                    [128, 8, NLOC], bf16, tag="L", bufs=2, name=f"XfT{s}"
                )
                XaT_s = bpool.tile(
                    [128, 8, NLOC], bf16, tag="L", bufs=2, name=f"XaT{s}"
                )
                emit_tpt(0)
                emit_u_pass(0)
                emit_post(2)
                emit_post(3)

                if s == 0:
                    # free the kp2 psum bank
                    nc.scalar.activation(KP2_s[:], kp2ps[:], Act.Copy)

                if s == 0:
                    # gather Xf' + kp2 in ONE AllGather: both only depend on
                    # the xf-part / step-0 A^T, so this launches before the
                    # xa matmuls and overlaps them.
                    for it in range(NT):
                        xf8 = stpool.tile([128, D], f8, tag="x2k", bufs=2, name="xf8")
                        nc.scalar.activation(xf8[:], Xf_loc[:, it, :], Act.Copy)
                        nc.sync.dma_start(
                            ag_m_in[it * 128 : (it + 1) * 128, :],
                            xf8[:].bitcast(f32),
                        )
                    nc.sync.dma_start(
                        ag_m_in[NLOC : NLOC + 128, :].bitcast(f32r),
                        KP2_s[0:64, :],
                    )
                    nc.gpsimd.collective_compute(
                        "AllGather",
                        Alu.bypass,
                        replica_groups=rg,
                        ins=[ag_m_in[:].opt()],
                        outs=[ag_m_out[:].opt()],
                    )

                # ---------- xa pair 1 + its transposes / U products
                emit_xa_pass(1, 0)
                emit_xa_pass(1, 1)

                if s == 0:
                    # stage + launch the Xa AllGather
                    for it in range(NT):
                        nc.sync.dma_start(
                            ag_xa_in[it * 128 : (it + 1) * 128, :], Xa_loc[:, it, :]
                        )
                    nc.gpsimd.collective_compute(
                        "AllGather",
                        Alu.bypass,
                        replica_groups=rg,
                        ins=[ag_xa_in[:].opt()],
                        outs=[ag_xa_out[:].opt()],
                    )

                emit_tpt(1)
                emit_u_pass(1)

                if s == 0:
                    # K2 two-half layout (waits on the merged AllGather) + local PT2
                    K2Th_s = rpool.tile(
                        [128, N // 2], f32r, tag="KTh", bufs=1, name="K2Th_s"
                    )
                    for r in range(NCORES):
                        ph = 0 if r < 4 else 64
                        off = r if r < 4 else r - 4
                        nc.sync.dma_start(
                            K2Th_s[ph : ph + 64, off * NLOC : (off + 1) * NLOC],
                            ag_m_out[r, NLOC : NLOC + 128, :].bitcast(f32r),
                        )
                    PT2_s = spool.tile(
                        [128, NLOC], f32r, tag="PT", bufs=2, name="PT2_s"
                    )
                    nc.sync.dma_start(PT2_s[0:64, :], KP2_s[64:128, :])
                    nc.sync.dma_start(PT2_s[64:128, :], KP2_s[64:128, :])

            # ================= LayerNorm epilogue =================
            if not ln_id:
                gamma_s1 = rpool.tile([1, D], f32, tag="K1P1", bufs=1, name="gamma_s1")
                beta_s1 = rpool.tile([1, D], f32, tag="KTh", bufs=1, name="beta_s1")
                nc.sync.dma_start(gamma_s1[:], gamma_d[:])
                nc.sync.dma_start(beta_s1[:], beta_d[:])
                gamma_bc = bpool.tile([128, D], f32, tag="XfT", bufs=2, name="gamma_bc")
                beta_bc = bpool.tile([128, D], f32, tag="XfT", bufs=2, name="beta_bc")
            for dh in range(2 if not ln_id else 0):
                gps = ps_tile(f"gps{dh}")
                nc.tensor.matmul(
                    gps[:],
                    ones1_s[:],
                    gamma_s1[:, dh * 512 : (dh + 1) * 512],
                    start=True,
                    stop=True,
                )
                nc.scalar.activation(
                    gamma_bc[:, dh * 512 : (dh + 1) * 512], gps[:], Act.Copy
                )
                bps = ps_tile(f"bps{dh}")
                nc.tensor.matmul(
                    bps[:],
                    ones1_s[:],
                    beta_s1[:, dh * 512 : (dh + 1) * 512],
                    start=True,
                    stop=True,
                )
                nc.scalar.activation(
                    beta_bc[:, dh * 512 : (dh + 1) * 512], bps[:], Act.Copy
                )

            inv_d = 1.0 / D
            for it in range(NT):
                ve = nc.vector
                Y = Z_s[:, it, :]          # Z_s already holds Z + X in f32
                sY = spool.tile([128, 1], f32, tag="sY", bufs=2, name="sY")
                nc.vector.tensor_reduce(sY[:], Y, mybir.AxisListType.X, Alu.add)
                scrB = bpool.tile([128, D], f32, tag="L", bufs=2, name="scrB")
                sY2 = spool.tile([128, 1], f32, tag="sY2", bufs=2, name="sY2")
                nc.scalar.activation(scrB[:], Y, Act.Square, accum_out=sY2[:])
                mu = spool.tile([128, 1], f32, tag="mu", bufs=2, name="mu")
                nc.vector.tensor_scalar_mul(mu[:], sY[:], inv_d)
                ex2 = spool.tile([128, 1], f32, tag="ex2", bufs=2, name="ex2")
                nc.vector.tensor_scalar_mul(ex2[:], sY2[:], inv_d)
                musq = spool.tile([128, 1], f32, tag="musq", bufs=2, name="musq")
                nc.vector.tensor_tensor(musq[:], mu[:], mu[:], Alu.mult)
                var = spool.tile([128, 1], f32, tag="var", bufs=2, name="var")
                nc.vector.tensor_tensor(var[:], ex2[:], musq[:], Alu.subtract)
                vpe = spool.tile([128, 1], f32, tag="vpe", bufs=2, name="vpe")
                nc.vector.tensor_scalar_add(vpe[:], var[:], LN_EPS)
                sd = spool.tile([128, 1], f32, tag="sd", bufs=2, name="sd")
                nc.scalar.activation(sd[:], vpe[:], Act.Sqrt)
                rstd = spool.tile([128, 1], f32, tag="rstd", bufs=2, name="rstd")
                nc.vector.reciprocal(rstd[:], sd[:])
                nmr = spool.tile([128, 1], f32, tag="nmr", bufs=2, name="nmr")
                nc.vector.tensor_tensor(nmr[:], mu[:], rstd[:], Alu.mult)
                nc.vector.tensor_scalar_mul(nmr[:], nmr[:], -1.0)
                tnorm = bpool.tile([128, D], f32, tag="L", bufs=2, name="tnorm")
                nc.scalar.activation(
                    tnorm[:], Y, Act.Identity, bias=nmr[:], scale=rstd[:]
                )
                if ln_id:
                    yout = tnorm
                else:
                    ve.tensor_tensor(tnorm[:], tnorm[:], gamma_bc[:], Alu.mult)
                    yout = bpool.tile([128, D], f32, tag="L", bufs=2, name="yout")
                    ve.tensor_tensor(yout[:], tnorm[:], beta_bc[:], Alu.add)
                nc.sync.dma_start(out_d[it * 128 : (it + 1) * 128, :], yout[:])

    nc.finalize()
    return nc


def _get_nc(ln_id=False):
    key = ("nc", ln_id)
    if key not in _CACHE:
        _CACHE[key] = _build(ln_id)
    return _CACHE[key]


def _ln_identity(inputs):
    g = np.asarray(inputs["gamma"], dtype=np.float32).ravel()
    b = np.asarray(inputs["beta"], dtype=np.float32).ravel()
    return bool(np.all(g == 1.0) and np.all(b == 0.0))


def _hilo16(x):
    # fp16 hi/lo split: x ~= hi + lo with hi = fp16(x), lo = fp16(x - hi)
    hi = np.ascontiguousarray(x, dtype=np.float32).astype(np.float16)
    lo = (x - hi.astype(np.float32)).astype(np.float16)
    return np.ascontiguousarray(hi), np.ascontiguousarray(lo)


def make_in_maps(inputs):
    import ml_dtypes

    bf = ml_dtypes.bfloat16
    X = np.ascontiguousarray(inputs["X"], dtype=np.float32)
    S = np.ascontiguousarray(inputs["S"], dtype=np.float32)
    W1 = np.asarray(inputs["W1"], dtype=np.float32)
    W2 = np.asarray(inputs["W2"], dtype=np.float32)
    W3 = np.asarray(inputs["W3"], dtype=np.float32)
    gamma = np.ascontiguousarray(inputs["gamma"], dtype=np.float32).reshape(1, D)
    beta = np.ascontiguousarray(inputs["beta"], dtype=np.float32).reshape(1, D)

    Xb = np.ascontiguousarray(X).astype(bf)
    Xb8 = np.ascontiguousarray(X).astype(ml_dtypes.float8_e4m3)
    W31 = np.concatenate([W3, W1 @ W2], axis=1)            # [D, 128]
    W31h, W31l = _hilo16(
        np.ascontiguousarray(W31.reshape(8, 128, 128).transpose(1, 0, 2))
    )
    Ub = {
        k: np.ascontiguousarray(inputs[k], dtype=np.float32).astype(bf)
        for k in ("U1_0", "U2_0", "U1_1", "U2_1")
    }

    in_maps = []
    for c in range(NCORES):
        lo, hi = c * NLOC, (c + 1) * NLOC
        Xloc = np.ascontiguousarray(X[lo:hi])
        XlocThi, XlocTlo = _hilo16(
            np.ascontiguousarray(Xloc.T.reshape(8, 128, NLOC).transpose(1, 0, 2))
        )
        # S + I prepacked: the diagonal ride-along makes Xf' = recipD*(S+I)@Xf
        # exact in one matmul stream (diag weight ~1/rowsum, f8 error there
        # is negligible)
        SpI = S[lo:hi].copy()
        SpI[np.arange(NLOC), lo + np.arange(NLOC)] += 1.0
        STb = np.ascontiguousarray(
            SpI.T.reshape(NJC, 128, NLOC).transpose(1, 0, 2)
        ).astype(ml_dtypes.float8_e4m3)
        m = {
            "Xb": Xb,
            "Xb8": Xb8,
            "Xloc": Xloc,
            "XlocThi": XlocThi,
            "XlocTlo": XlocTlo,
            "STb": STb,
            "W31h": W31h,
            "W31l": W31l,
            "gamma": gamma,
            "beta": beta,
        }
        m.update(Ub)
        in_maps.append(m)
    return in_maps


def kernel(**inputs):
    from concourse.bass_utils import run_bass_kernel_spmd

    nc = _get_nc(_ln_identity(inputs))
    in_maps = make_in_maps(inputs)
    res = run_bass_kernel_spmd(nc, in_maps, core_ids=list(range(NCORES)))
    out = np.concatenate([res.results[c]["out"] for c in range(NCORES)], axis=0)
    return np.ascontiguousarray(out, dtype=np.float32)



# revision 68
# speedup vs baseline: 1.0355x; 1.0355x over previous
"""Trainium2 Bass kernel for nn_AdaptiveFunctionBlock (gnn_message_passing).

Row-shards N=4096 across 8 NeuronCores (512 rows each).  Host prepacks
bf16 / pre-transposed views of the inputs (X, X^T, S_loc^T, [W3|W1@W2],
U_s) so the device does no layout shuffles or casts of the raw inputs.
S^T and A^T live in SBUF for the whole kernel (no HBM scratch).

Per core:
  prologue: rowsums of S (PE ones-matmul), KP^T = [W3|W1W2]^T @ X^T
    (full, so no K/P AllGather), K1P1 row-major via PE transposes.
  step s: L = P@K^T (bf16 matmuls, f32 psum), exact per-row top-p via
    segment top-8 candidates -> sorted top-T -> f32 cumsum threshold
    (reference-faithful tie handling on step 2), softmax -> A (bf16),
    A^T via PE transposes into resident SBUF, then row-parallel GEMMs
    Xa' = A@Xa, Xf' = recipD*(S@Xf + Xf_diag), Z += Xf'@U1_s + Xa'@U2_s,
    with an AllGather of (Xf'|Xa') between steps and a small AllGather
    of K2P2 for the step-2 logits.  Finally LayerNorm(X+Z)*gamma+beta.
"""

import numpy as np

N = 4096
D = 1024
DA = 64
NCORES = 8
NLOC = N // NCORES          # 512
NT = NLOC // 128            # 4 row-tiles per core
NJC = N // 128              # 32 column chunks
P_TOP = 0.9
LN_EPS = 1e-5
T1, T2 = 16, 96             # top-p extraction depth per step
NSEG = 32
SEGW = N // NSEG            # 128
NEGINF = -3.0e38
POSINF = 3.0e38

_CACHE = {}


def _build(ln_id=False):
    import concourse.bass as bass
    import concourse.mybir as mybir
    from concourse import bacc, tile

    dt = mybir.dt
    f32 = dt.float32
    f32r = dt.float32r
    bf16 = dt.bfloat16
    f8 = dt.float8e4
    DR = mybir.MatmulPerfMode.DoubleRow
    Act = mybir.ActivationFunctionType
    Alu = mybir.AluOpType
    PSUM = bass.MemorySpace.PSUM
    DRAM = bass.MemorySpace.DRAM

    nc = bacc.Bacc(num_devices=NCORES)

    # ---------------- I/O ----------------
    f16 = dt.float16
    Xb_d = nc.dram_tensor("Xb", [N, D], bf16, kind="ExternalInput")
    Xb8_d = nc.dram_tensor("Xb8", [N, D], f8, kind="ExternalInput")
    Xloc_d = nc.dram_tensor("Xloc", [NLOC, D], f32, kind="ExternalInput")
    XlocThi_d = nc.dram_tensor("XlocThi", [128, 8, NLOC], f16, kind="ExternalInput")
    XlocTlo_d = nc.dram_tensor("XlocTlo", [128, 8, NLOC], f16, kind="ExternalInput")
    STb_d = nc.dram_tensor("STb", [128, NJC, NLOC], f8, kind="ExternalInput")
    W31h_d = nc.dram_tensor("W31h", [128, 8, 128], f16, kind="ExternalInput")
    W31l_d = nc.dram_tensor("W31l", [128, 8, 128], f16, kind="ExternalInput")
    U_d = [
        nc.dram_tensor("U1_0", [D, D], bf16, kind="ExternalInput"),
        nc.dram_tensor("U2_0", [D, D], bf16, kind="ExternalInput"),
        nc.dram_tensor("U1_1", [D, D], bf16, kind="ExternalInput"),
        nc.dram_tensor("U2_1", [D, D], bf16, kind="ExternalInput"),
    ]
    gamma_d = nc.dram_tensor("gamma", [1, D], f32, kind="ExternalInput")
    beta_d = nc.dram_tensor("beta", [1, D], f32, kind="ExternalInput")
    out_d = nc.dram_tensor("out", [NLOC, D], f32, kind="ExternalOutput")

    # ---------------- inline constants ----------------
    identb_c = nc.inline_tensor(
        np.eye(128, dtype=np.float32), name="identb_c"
    )
    ones1_c = nc.inline_tensor(np.ones((1, 128), dtype=np.float32), name="ones1_c")
    onesp_c = nc.inline_tensor(np.ones((128, 1), dtype=np.float32), name="onesp_c")

    rg = [list(range(NCORES))]
    Tmax = max(T1, T2)

    with tile.TileContext(nc) as tc:
        with (
            tc.tile_pool(name="const", bufs=1) as cpool,
            tc.tile_pool(name="res", bufs=1) as rpool,
            tc.tile_pool(name="big", bufs=1) as bpool,
            tc.tile_pool(name="stream", bufs=1) as stpool,
            tc.tile_pool(name="small", bufs=1) as spool,
            tc.tile_pool(name="psum", bufs=1, space=PSUM) as ppool,
            tc.tile_pool(name="dram", bufs=1, space=DRAM) as dpool,
        ):
            def ps_tile(name):
                return ppool.tile([128, 512], f32, tag="pb", bufs=8, name=name)

            def ps_tile_b(name):
                return ppool.tile([128, 512], bf16, tag="pb", bufs=8, name=name)

            # ---------- SBUF consts ----------
            identf_s = cpool.tile([128, 128], f32, name="identf_s")
            nc.sync.dma_start(identf_s[:], identb_c[:])
            identr_s = cpool.tile([128, 128], f32r, name="identr_s")
            nc.sync.dma_start(identr_s[:], identb_c[:].bitcast(f32r))
            identb_s = cpool.tile([128, 128], bf16, name="identb_s")
            nc.vector.tensor_copy(identb_s[:], identf_s[:])
            ones1_s = cpool.tile([1, 128], f32, name="ones1_s")
            nc.sync.dma_start(ones1_s[:], ones1_c[:])
            onespf_s = cpool.tile([128, 1], f32, name="onespf_s")
            nc.sync.dma_start(onespf_s[:], onesp_c[:])
            onespb_s = cpool.tile([128, 1], f8, name="onespb_s")
            nc.vector.tensor_copy(onespb_s[:], onespf_s[:])
            zerosT_s = cpool.tile([128, Tmax], f32, name="zerosT_s")
            nc.vector.memset(zerosT_s[:], 0.0)

            # ---------- collective DRAM buffers ----------
            # merged AllGather payload: Xf' (NLOC x D f8 viewed as f32) then
            # kp2 (64 x NLOC f32 flattened into 128 rows of 256)
            ag_m_in = dpool.tile([NLOC + 128, 256], f32, name="ag_m_in")
            ag_m_out = dpool.tile(
                [NCORES, NLOC + 128, 256], f32, addr_space="Shared", name="ag_m_out"
            )
            # prologue KP^T AllGather: each core contributes its local cols
            ag_kp1_in = dpool.tile([128, NLOC], f32, name="ag_kp1_in")
            ag_kp1_out = dpool.tile(
                [NCORES, 128, NLOC], f32, addr_space="Shared", name="ag_kp1_out"
            )
            ag_xa_in = dpool.tile([NLOC, D], bf16, name="ag_xa_in")
            ag_xa_out = dpool.tile(
                [NCORES, NLOC, D], bf16, addr_space="Shared", name="ag_xa_out"
            )
            # dummy warm-up AllGather: absorbs the one-time comm-init
            # BARRIER while the prologue computes (no data deps)
            ag_w_in = dpool.tile([1, 64], f32, name="ag_w_in")
            ag_w_out = dpool.tile(
                [NCORES, 1, 64], f32, addr_space="Shared", name="ag_w_out"
            )
            nc.gpsimd.collective_compute(
                "AllGather",
                Alu.bypass,
                replica_groups=rg,
                ins=[ag_w_in[:].opt()],
                outs=[ag_w_out[:].opt()],
            )

            # ---------- residents ----------
            recipD_s = rpool.tile([128, NT], f32, name="recipD_s")
            Z_s = rpool.tile([128, NT, D], f32, name="Z_s")
            ST_s = rpool.tile([128, NJC, NLOC], f8, name="ST_s")
            ATb_s = rpool.tile([128, NT, N], bf16, name="ATb_s")
            K1P1rm_s = rpool.tile([128, NJC * 128], bf16, tag="K1P1", bufs=1, name="K1P1rm_s")
            Xf_loc = rpool.tile([128, NT, D], bf16, name="Xf_loc")
            Xa_loc = rpool.tile([128, NT, D], bf16, name="Xa_loc")
            KP2_s = rpool.tile([128, NLOC], f32r, name="KP2_s")

            # ================= PROLOGUE =================
            # S^T resident (prepacked on host, fully contiguous load)
            nc.sync.dma_start(ST_s[:], STb_d[:])
            W31h_s = stpool.tile([128, 8, 128], f16, tag="x4k", bufs=2, name="W31h_s")
            nc.sync.dma_start(W31h_s[:], W31h_d[:])
            W31l_s = stpool.tile([128, 8, 128], f16, tag="x4k", bufs=2, name="W31l_s")
            nc.sync.dma_start(W31l_s[:], W31l_d[:])

            # local KP^T = W31^T @ Xloc^T  (fp16 hi/lo 3-pass: ~exact), then
            # AllGather the 512-col slices so every core gets full KP^T.
            xlh = bpool.tile([128, 8, NLOC], f16, tag="L", bufs=2, name="xlh")
            nc.sync.dma_start(xlh[:], XlocThi_d[:])
            xll = bpool.tile([128, 8, NLOC], f16, tag="L", bufs=2, name="xll")
            nc.sync.dma_start(xll[:], XlocTlo_d[:])
            kqlp = ps_tile("kqlp")
            for dc in range(8):
                nc.tensor.matmul(
                    kqlp[:], W31h_s[:, dc, :], xlh[:, dc, :],
                    start=(dc == 0), stop=False,
                )
                nc.tensor.matmul(
                    kqlp[:], W31l_s[:, dc, :], xlh[:, dc, :], start=False, stop=False
                )
                nc.tensor.matmul(
                    kqlp[:], W31h_s[:, dc, :], xll[:, dc, :],
                    start=False, stop=(dc == 7),
                )
            KPloc_s = spool.tile([128, NLOC], f32, tag="PT", bufs=2, name="KPloc_s")
            nc.scalar.activation(KPloc_s[:], kqlp[:], Act.Copy)
            PT1_s = spool.tile([128, NLOC], f32, tag="PT", bufs=2, name="PT1_s")
            nc.sync.dma_start(PT1_s[0:64, :], KPloc_s[64:128, :])
            nc.sync.dma_start(PT1_s[64:128, :], KPloc_s[64:128, :])
            nc.sync.dma_start(ag_kp1_in[:], KPloc_s[:])
            nc.gpsimd.collective_compute(
                "AllGather",
                Alu.bypass,
                replica_groups=rg,
                ins=[ag_kp1_in[:].opt()],
                outs=[ag_kp1_out[:].opt()],
            )

            # assemble K1 two-half layout + full KP^T from the gather
            KPT_s = bpool.tile([128, N], f32, tag="E", bufs=1, name="KPT_s")
            K1Th_s = rpool.tile([128, N // 2], f32, tag="KTh", bufs=1, name="K1Th_s")
            for co in range(8):
                nc.sync.dma_start(
                    KPT_s[:, co * 512 : (co + 1) * 512], ag_kp1_out[co, :, :]
                )
                ph = 0 if co < 4 else 64
                off = co if co < 4 else co - 4
                nc.sync.dma_start(
                    K1Th_s[ph : ph + 64, off * 512 : (off + 1) * 512],
                    ag_kp1_out[co, 0:64, :],
                )

            # K1P1 row-major bf16 (for K2P2 accumulation) via f32 PE transposes
            for jq in range(8):
                tpk = ps_tile(f"tpk{jq}")
                for b in range(4):
                    jc = jq * 4 + b
                    nc.tensor.transpose(
                        tpk[:, b * 128 : (b + 1) * 128],
                        KPT_s[:, jc * 128 : (jc + 1) * 128],
                        identf_s[:],
                    )
                nc.scalar.activation(
                    K1P1rm_s[:, jq * 512 : (jq + 1) * 512], tpk[:], Act.Copy
                )

            kp2ps = None
            K2Th_s = None
            PT2_s = None

            # ================= STEPS =================
            for s in range(2):
                T = T1 if s == 0 else T2
                if s == 0:
                    PTl, KTh = PT1_s, K1Th_s
                else:
                    PTl, KTh = PT2_s, K2Th_s
                Ab_t = [None] * NT

                def emit_post(it, s=s):
                    # A^T into resident SBUF (bf16 PE transposes), then the
                    # step-0 K2P2 accumulation for this row tile.
                    Ab = Ab_t[it]
                    for jq in range(8):
                        tpa = ps_tile_b(f"tpa{s}{it}{jq}")
                        for b in range(4):
                            jc = jq * 4 + b
                            nc.tensor.transpose(
                                tpa[:, b * 128 : (b + 1) * 128],
                                Ab[:, jc * 128 : (jc + 1) * 128],
                                identb_s[:],
                            )
                        nc.scalar.activation(
                            ATb_s[:, it, jq * 512 : (jq + 1) * 512], tpa[:], Act.Copy
                        )
                    if s == 0:
                        for jc in range(NJC):
                            nc.tensor.matmul(
                                kp2ps[:, it * 128 : (it + 1) * 128],
                                K1P1rm_s[:, jc * 128 : (jc + 1) * 128],
                                ATb_s[:, it, jc * 128 : (jc + 1) * 128],
                                start=(jc == 0),
                                stop=(jc == NJC - 1),
                            )

                # xf-part: pair-split passes, 2 psum banks live at a time
                xfp = {}

                def emit_xf_pass(pr, dh, s=s):
                    xfp[(pr, dh)] = [
                        ps_tile(f"xfp{s}{pr}{dh}a"),
                        ps_tile(f"xfp{s}{pr}{dh}b"),
                    ]
                    for q2 in range(NJC // 2):
                        xch = stpool.tile(
                            [128, 2, 512], f8, tag="xch5", bufs=3, name="xch"
                        )
                        for kt in range(2):
                            jc = 2 * q2 + kt
                            if s == 0:
                                nc.sync.dma_start(
                                    xch[:, kt, :],
                                    Xb8_d[
                                        jc * 128 : (jc + 1) * 128,
                                        dh * 512 : (dh + 1) * 512,
                                    ],
                                )
                            else:
                                r_, blk_ = jc // NT, jc % NT
                                nc.sync.dma_start(
                                    xch[:, kt, :],
                                    ag_m_out[
                                        r_,
                                        blk_ * 128 : (blk_ + 1) * 128,
                                        dh * 128 : (dh + 1) * 128,
                                    ].bitcast(f8),
                                )
                        for j in range(2):
                            itt = 2 * pr + j
                            nc.tensor.matmul(
                                xfp[(pr, dh)][j][:],
                                ST_s[:, 2 * q2 : 2 * q2 + 2, itt * 128 : (itt + 1) * 128],
                                xch[:],
                                start=(q2 == 0),
                                stop=(q2 == NJC // 2 - 1),
                                perf_mode=DR,
                            )
                    # Xf' = recipD * psum (S+I is prepacked, diag included);
                    # frees the pass banks
                    for j in range(2):
                        it = 2 * pr + j
                        nc.vector.tensor_scalar_mul(
                            Xf_loc[:, it, dh * 512 : (dh + 1) * 512],
                            xfp[(pr, dh)][j][:],
                            recipD_s[:, it : it + 1],
                        )

                def emit_xa_pair(pr, s=s):
                    # one full-width stream for the pair: 4 psum banks
                    xab = [
                        [ps_tile(f"xap{s}{pr}{j}{dh}") for dh in range(2)]
                        for j in range(2)
                    ]
                    for jc in range(NJC):
                        xch = stpool.tile(
                            [128, D], bf16, tag="xchw", bufs=2, name="xcha"
                        )
                        if s == 0:
                            nc.sync.dma_start(
                                xch[:], Xb_d[jc * 128 : (jc + 1) * 128, :]
                            )
                        else:
                            r_, blk_ = jc // NT, jc % NT
                            nc.sync.dma_start(
                                xch[:],
                                ag_xa_out[r_, blk_ * 128 : (blk_ + 1) * 128, :],
                            )
                        for j in range(2):
                            it = 2 * pr + j
                            for dh in range(2):
                                nc.tensor.matmul(
                                    xab[j][dh][:],
                                    ATb_s[:, it, jc * 128 : (jc + 1) * 128],
                                    xch[:, dh * 512 : (dh + 1) * 512],
                                    start=(jc == 0),
                                    stop=(jc == NJC - 1),
                                )
                    for j in range(2):
                        it = 2 * pr + j
                        for dh in range(2):
                            nc.scalar.activation(
                                Xa_loc[:, it, dh * 512 : (dh + 1) * 512],
                                xab[j][dh][:],
                                Act.Copy,
                            )

                def emit_tpt(s=s):
                    # Xf^T / Xa^T (all tiles)
                    for src, dst in ((Xf_loc, XfT_s), (Xa_loc, XaT_s)):
                        for dc in range(8):
                            tpt = ps_tile_b(f"tpt{s}{dc}")
                            for it in range(NT):
                                nc.tensor.transpose(
                                    tpt[:, it * 128 : (it + 1) * 128],
                                    src[:, it, dc * 128 : (dc + 1) * 128],
                                    identb_s[:],
                                )
                            nc.scalar.activation(dst[:, dc, :], tpt[:], Act.Copy)

                def emit_u_all(s=s):
                    # Z = Xf'@U1_s + Xa'@U2_s (+ X on step 0): 8 psum banks
                    zp = [[ps_tile(f"zp{s}{it}{dh}") for dh in range(2)]
                          for it in range(NT)]
                    for u, XT in ((0, XfT_s), (1, XaT_s)):
                        for dc in range(8):
                            ubf = stpool.tile(
                                [128, D], bf16, tag="x2k", bufs=2, name="ubf"
                            )
                            nc.sync.dma_start(
                                ubf[:], U_d[2 * s + u][dc * 128 : (dc + 1) * 128, :]
                            )
                            for it in range(NT):
                                for dh in range(2):
                                    nc.tensor.matmul(
                                        zp[it][dh][:],
                                        XT[:, dc, it * 128 : (it + 1) * 128],
                                        ubf[:, dh * 512 : (dh + 1) * 512],
                                        start=(u == 0 and dc == 0),
                                        stop=(u == 1 and dc == 7),
                                    )
                    for it in range(NT):
                        for dh in range(2):
                            if s == 0:
                                xlf = stpool.tile(
                                    [128, 512], f32, tag="xth", bufs=2, name="xlf0"
                                )
                                nc.sync.dma_start(
                                    xlf[:],
                                    Xloc_d[
                                        it * 128 : (it + 1) * 128,
                                        dh * 512 : (dh + 1) * 512,
                                    ],
                                )
                                nc.vector.tensor_tensor(
                                    Z_s[:, it, dh * 512 : (dh + 1) * 512],
                                    zp[it][dh][:],
                                    xlf[:],
                                    Alu.add,
                                )
                            else:
                                nc.vector.tensor_tensor(
                                    Z_s[:, it, dh * 512 : (dh + 1) * 512],
                                    Z_s[:, it, dh * 512 : (dh + 1) * 512],
                                    zp[it][dh][:],
                                    Alu.add,
                                )

                def emit_lp(it, s=s):
                    L = bpool.tile([128, N], f32, tag="L", bufs=2, name="L")
                    for jc8 in range(8):
                        lp = ps_tile(f"lp{it}{jc8}")
                        kph = 0 if jc8 < 4 else 64
                        kpo = jc8 if jc8 < 4 else jc8 - 4
                        nc.tensor.matmul(
                            lp[:],
                            PTl[kph : kph + 64, it * 128 : (it + 1) * 128],
                            KTh[kph : kph + 64, kpo * 512 : (kpo + 1) * 512],
                            start=True,
                            stop=True,
                        )
                        nc.scalar.activation(
                            L[:, jc8 * 512 : (jc8 + 1) * 512], lp[:], Act.Copy
                        )
                    return L

                if s == 0:
                    # rowsums of S (+1 for the identity) -> recipD
                    kp2ps = ps_tile("kp2ps")
                    for rit in range(NT):
                        rsp = ps_tile(f"rsp{rit}")
                        for jc in range(NJC):
                            nc.tensor.matmul(
                                rsp[:, 0:1],
                                ST_s[:, jc, rit * 128 : (rit + 1) * 128],
                                onespb_s[:],
                                start=(jc == 0),
                                stop=(jc == NJC - 1),
                            )
                        nc.vector.reciprocal(
                            recipD_s[:, rit : rit + 1], rsp[:, 0:1]
                        )

                L_t = [None] * NT
                L_t[0] = emit_lp(0)
                L_t[1] = emit_lp(1)
                emit_xf_pass(0, 0)
                emit_xf_pass(0, 1)
                emit_xf_pass(1, 0)
                emit_xf_pass(1, 1)

                def emit_topp(it, s=s, T=T):
                    L = L_t[it]
                    # --- candidates: per-segment top-8
                    cand = spool.tile([128, 256], f32, tag="cand", bufs=2, name="cand")
                    for sg in range(NSEG):
                        nc.vector.max(
                            cand[:, sg * 8 : sg * 8 + 8],
                            L[:, sg * SEGW : (sg + 1) * SEGW],
                        )
                    if s == 1:
                        cand_copy = spool.tile(
                            [128, 256], f32, tag="cand", bufs=2, name="cand_copy"
                        )
                        nc.vector.tensor_copy(cand_copy[:], cand[:])

                    # --- extract sorted top-T (destroys cand)
                    V = spool.tile([128, Tmax], f32, tag="V", bufs=2, name="V")
                    for rnd in range(T // 8):
                        nc.vector.max(V[:, rnd * 8 : rnd * 8 + 8], cand[:])
                        if rnd < T // 8 - 1:
                            nc.vector.match_replace(
                                cand[:], V[:, rnd * 8 : rnd * 8 + 8], cand[:], NEGINF
                            )

                    negm = spool.tile([128, 1], f32, tag="negm", bufs=2, name="negm")
                    nc.vector.tensor_scalar_mul(negm[:], V[:, 0:1], -1.0)

                    E = bpool.tile([128, N], f32, tag="E", bufs=1, name="E")
                    Zrow = spool.tile([128, 1], f32, tag="Zrow", bufs=2, name="Zrow")
                    nc.scalar.activation(
                        E[:], L[:], Act.Exp, bias=negm[:], accum_out=Zrow[:]
                    )
                    EV = spool.tile([128, Tmax], f32, tag="EV", bufs=2, name="EV")
                    nc.scalar.activation(EV[:, 0:T], V[:, 0:T], Act.Exp, bias=negm[:])
                    cs = spool.tile([128, Tmax], f32, tag="cs", bufs=2, name="cs")
                    nc.vector.tensor_tensor_scan(
                        cs[:, 0:T], EV[:, 0:T], zerosT_s[:, 0:T], 0.0, Alu.add, Alu.add
                    )
                    thr = spool.tile([128, 1], f32, tag="thr", bufs=2, name="thr")
                    nc.vector.tensor_scalar_mul(thr[:], Zrow[:], P_TOP)
                    kept = spool.tile([128, Tmax], f32, tag="kept", bufs=2, name="kept")
                    nc.vector.scalar_tensor_tensor(
                        kept[:, 0:T], cs[:, 0:T], thr[:], EV[:, 0:T],
                        Alu.subtract, Alu.is_lt,
                    )
                    scr1 = spool.tile([128, Tmax], f32, tag="scr1", bufs=1, name="scr1")
                    Drow = spool.tile([128, 1], f32, tag="Drow", bufs=2, name="Drow")
                    nc.vector.tensor_tensor(
                        scr1[:, 0:T], EV[:, 0:T], kept[:, 0:T], Alu.mult
                    )
                    nc.vector.tensor_reduce(
                        Drow[:], scr1[:, 0:T], mybir.AxisListType.X, Alu.add
                    )
                    nki = spool.tile([128, Tmax], f32, tag="nki", bufs=1, name="nki")
                    nc.vector.tensor_scalar(
                        nki[:, 0:T], kept[:, 0:T], 0.5, POSINF, Alu.is_lt, Alu.mult
                    )
                    scr2 = spool.tile([128, Tmax], f32, tag="scr2", bufs=1, name="scr2")
                    tau = spool.tile([128, 1], f32, tag="tau", bufs=2, name="tau")
                    nc.vector.tensor_tensor(
                        scr2[:, 0:T], nki[:, 0:T], V[:, 0:T], Alu.add
                    )
                    nc.vector.tensor_reduce(
                        tau[:], scr2[:, 0:T], mybir.AxisListType.X, Alu.min
                    )
                    recD = spool.tile([128, 1], f32, tag="recD", bufs=2, name="recD")
                    nc.vector.reciprocal(recD[:], Drow[:])
                    etau = spool.tile([128, 1], f32, tag="etau", bufs=2, name="etau")
                    nc.scalar.activation(etau[:], tau[:], Act.Exp, bias=negm[:])

                    if s == 1:
                        scrT = spool.tile(
                            [128, Tmax], f32, tag="scrT", bufs=1, name="scrT"
                        )
                        rr = spool.tile([128, 1], f32, tag="rr", bufs=2, name="rr")
                        nc.vector.scalar_tensor_tensor(
                            scrT[:, 0:T], V[:, 0:T], tau[:], kept[:, 0:T],
                            Alu.is_equal, Alu.mult,
                        )
                        nc.vector.tensor_reduce(
                            rr[:], scrT[:, 0:T], mybir.AxisListType.X, Alu.add
                        )
                        scr256 = spool.tile(
                            [128, 256], f32, tag="scrT", bufs=1, name="scr256"
                        )
                        ceq = spool.tile([128, 1], f32, tag="ceq", bufs=2, name="ceq")
                        nc.vector.tensor_scalar(
                            scr256[:], cand_copy[:], tau[:], None, Alu.is_equal
                        )
                        nc.vector.tensor_reduce(
                            ceq[:], scr256[:], mybir.AxisListType.X, Alu.add
                        )
                        # w2 = (ceq - r)/ceq * etau * recD  (spread over all ties)
                        wv = spool.tile([128, 1], f32, tag="wv", bufs=2, name="wv")
                        nc.vector.tensor_tensor(wv[:], ceq[:], rr[:], Alu.subtract)
                        nc.vector.tensor_tensor(wv[:], wv[:], etau[:], Alu.mult)
                        nc.vector.tensor_tensor(wv[:], wv[:], recD[:], Alu.mult)
                        rceq = spool.tile([128, 1], f32, tag="rceq", bufs=2, name="rceq")
                        nc.vector.reciprocal(rceq[:], ceq[:])
                        nc.vector.tensor_tensor(wv[:], wv[:], rceq[:], Alu.mult)

                    # --- A materialization: E := (E >= etau) * E;  Ab = E*recD
                    # (step 2 fuses the tie-drop correction into the scale)
                    nc.vector.scalar_tensor_tensor(
                        E[:], E[:], etau[:], E[:], Alu.is_ge, Alu.mult
                    )
                    Ab = bpool.tile([128, N], bf16, tag="XfT", bufs=2, name="Ab")
                    if s == 1:
                        eqw = rpool.tile([128, N], bf16, tag="K1P1", bufs=1, name="eqw")
                        nc.vector.tensor_scalar(
                            eqw[:], L[:], tau[:], wv[:], Alu.is_equal, Alu.mult
                        )
                        nc.vector.scalar_tensor_tensor(
                            Ab[:], E[:], recD[:], eqw[:], Alu.mult, Alu.subtract
                        )
                    else:
                        nc.scalar.activation(Ab[:], E[:], Act.Copy, scale=recD[:])
                    Ab_t[it] = Ab

                emit_topp(0)
                emit_topp(1)
                L_t[2] = emit_lp(2)
                L_t[3] = emit_lp(3)
                emit_post(0)
                emit_post(1)
                emit_xa_pair(0)
                emit_topp(2)
                emit_topp(3)
                emit_post(2)
                emit_post(3)
                XfT_s = bpool.tile(
                    [128, 8, NLOC], bf16, tag="L", bufs=2, name=f"XfT{s}"
                )
                XaT_s = bpool.tile(
                    [128, 8, NLOC], bf16, tag="L", bufs=2, name=f"XaT{s}"
                )

                if s == 0:
                    # free the kp2 psum bank
                    nc.scalar.activation(KP2_s[:], kp2ps[:], Act.Copy)

                if s == 0:
                    # gather Xf' + kp2 in ONE AllGather: both only depend on
                    # the xf-part / step-0 A^T, so this launches before the
                    # xa matmuls and overlaps them.
                    for it in range(NT):
                        xf8 = stpool.tile([128, D], f8, tag="x2k", bufs=2, name="xf8")
                        nc.scalar.activation(xf8[:], Xf_loc[:, it, :], Act.Copy)
                        nc.sync.dma_start(
                            ag_m_in[it * 128 : (it + 1) * 128, :],
                            xf8[:].bitcast(f32),
                        )
                    nc.sync.dma_start(
                        ag_m_in[NLOC : NLOC + 128, :].bitcast(f32r),
                        KP2_s[0:64, :],
                    )
                    nc.gpsimd.collective_compute(
                        "AllGather",
                        Alu.bypass,
                        replica_groups=rg,
                        ins=[ag_m_in[:].opt()],
                        outs=[ag_m_out[:].opt()],
                    )

                # ---------- xa pair 1, then transposes + U products
                emit_xa_pair(1)

                if s == 0:
                    # stage + launch the Xa AllGather
                    for it in range(NT):
                        nc.sync.dma_start(
                            ag_xa_in[it * 128 : (it + 1) * 128, :], Xa_loc[:, it, :]
                        )
                    nc.gpsimd.collective_compute(
                        "AllGather",
                        Alu.bypass,
                        replica_groups=rg,
                        ins=[ag_xa_in[:].opt()],
                        outs=[ag_xa_out[:].opt()],
                    )

                emit_tpt()
                emit_u_all()

                if s == 0:
                    # K2 two-half layout (waits on the merged AllGather) + local PT2
                    K2Th_s = rpool.tile(
                        [128, N // 2], f32r, tag="KTh", bufs=1, name="K2Th_s"
                    )
                    for r in range(NCORES):
                        ph = 0 if r < 4 else 64
                        off = r if r < 4 else r - 4
                        nc.sync.dma_start(
                            K2Th_s[ph : ph + 64, off * NLOC : (off + 1) * NLOC],
                            ag_m_out[r, NLOC : NLOC + 128, :].bitcast(f32r),
                        )
                    PT2_s = spool.tile(
                        [128, NLOC], f32r, tag="PT", bufs=2, name="PT2_s"
                    )
                    nc.sync.dma_start(PT2_s[0:64, :], KP2_s[64:128, :])
                    nc.sync.dma_start(PT2_s[64:128, :], KP2_s[64:128, :])

            # ================= LayerNorm epilogue =================
            if not ln_id:
                gamma_s1 = rpool.tile([1, D], f32, tag="K1P1", bufs=1, name="gamma_s1")
                beta_s1 = rpool.tile([1, D], f32, tag="KTh", bufs=1, name="beta_s1")
                nc.sync.dma_start(gamma_s1[:], gamma_d[:])
                nc.sync.dma_start(beta_s1[:], beta_d[:])
                gamma_bc = bpool.tile([128, D], f32, tag="XfT", bufs=2, name="gamma_bc")
                beta_bc = bpool.tile([128, D], f32, tag="XfT", bufs=2, name="beta_bc")
            for dh in range(2 if not ln_id else 0):
                gps = ps_tile(f"gps{dh}")
                nc.tensor.matmul(
                    gps[:],
                    ones1_s[:],
                    gamma_s1[:, dh * 512 : (dh + 1) * 512],
                    start=True,
                    stop=True,
                )
                nc.scalar.activation(
                    gamma_bc[:, dh * 512 : (dh + 1) * 512], gps[:], Act.Copy
                )
                bps = ps_tile(f"bps{dh}")
                nc.tensor.matmul(
                    bps[:],
                    ones1_s[:],
                    beta_s1[:, dh * 512 : (dh + 1) * 512],
                    start=True,
                    stop=True,
                )
                nc.scalar.activation(
                    beta_bc[:, dh * 512 : (dh + 1) * 512], bps[:], Act.Copy
                )

            inv_d = 1.0 / D
            for it in range(NT):
                ve = nc.vector
                Y = Z_s[:, it, :]          # Z_s already holds Z + X in f32
                sY = spool.tile([128, 1], f32, tag="sY", bufs=2, name="sY")
                nc.vector.tensor_reduce(sY[:], Y, mybir.AxisListType.X, Alu.add)
                scrB = bpool.tile([128, D], f32, tag="L", bufs=2, name="scrB")
                sY2 = spool.tile([128, 1], f32, tag="sY2", bufs=2, name="sY2")
                nc.scalar.activation(scrB[:], Y, Act.Square, accum_out=sY2[:])
                mu = spool.tile([128, 1], f32, tag="mu", bufs=2, name="mu")
                nc.vector.tensor_scalar_mul(mu[:], sY[:], inv_d)
                ex2 = spool.tile([128, 1], f32, tag="ex2", bufs=2, name="ex2")
                nc.vector.tensor_scalar_mul(ex2[:], sY2[:], inv_d)
                musq = spool.tile([128, 1], f32, tag="musq", bufs=2, name="musq")
                nc.vector.tensor_tensor(musq[:], mu[:], mu[:], Alu.mult)
                var = spool.tile([128, 1], f32, tag="var", bufs=2, name="var")
                nc.vector.tensor_tensor(var[:], ex2[:], musq[:], Alu.subtract)
                vpe = spool.tile([128, 1], f32, tag="vpe", bufs=2, name="vpe")
                nc.vector.tensor_scalar_add(vpe[:], var[:], LN_EPS)
                sd = spool.tile([128, 1], f32, tag="sd", bufs=2, name="sd")
                nc.scalar.activation(sd[:], vpe[:], Act.Sqrt)
                rstd = spool.tile([128, 1], f32, tag="rstd", bufs=2, name="rstd")
                nc.vector.reciprocal(rstd[:], sd[:])
                nmr = spool.tile([128, 1], f32, tag="nmr", bufs=2, name="nmr")
                nc.vector.tensor_tensor(nmr[:], mu[:], rstd[:], Alu.mult)
                nc.vector.tensor_scalar_mul(nmr[:], nmr[:], -1.0)
                tnorm = bpool.tile([128, D], f32, tag="L", bufs=2, name="tnorm")
                nc.scalar.activation(
                    tnorm[:], Y, Act.Identity, bias=nmr[:], scale=rstd[:]
                )
                if ln_id:
                    yout = tnorm
                else:
                    ve.tensor_tensor(tnorm[:], tnorm[:], gamma_bc[:], Alu.mult)
                    yout = bpool.tile([128, D], f32, tag="L", bufs=2, name="yout")
                    ve.tensor_tensor(yout[:], tnorm[:], beta_bc[:], Alu.add)
                nc.sync.dma_start(out_d[it * 128 : (it + 1) * 128, :], yout[:])

    nc.finalize()
    return nc


def _get_nc(ln_id=False):
    key = ("nc", ln_id)
    if key not in _CACHE:
        _CACHE[key] = _build(ln_id)
    return _CACHE[key]


def _ln_identity(inputs):
    g = np.asarray(inputs["gamma"], dtype=np.float32).ravel()
    b = np.asarray(inputs["beta"], dtype=np.float32).ravel()
    return bool(np.all(g == 1.0) and np.all(b == 0.0))


def _hilo16(x):
    # fp16 hi/lo split: x ~= hi + lo with hi = fp16(x), lo = fp16(x - hi)
    hi = np.ascontiguousarray(x, dtype=np.float32).astype(np.float16)
    lo = (x - hi.astype(np.float32)).astype(np.float16)
    return np.ascontiguousarray(hi), np.ascontiguousarray(lo)


def make_in_maps(inputs):
    import ml_dtypes

    bf = ml_dtypes.bfloat16
    X = np.ascontiguousarray(inputs["X"], dtype=np.float32)
    S = np.ascontiguousarray(inputs["S"], dtype=np.float32)
    W1 = np.asarray(inputs["W1"], dtype=np.float32)
    W2 = np.asarray(inputs["W2"], dtype=np.float32)
    W3 = np.asarray(inputs["W3"], dtype=np.float32)
    gamma = np.ascontiguousarray(inputs["gamma"], dtype=np.float32).reshape(1, D)
    beta = np.ascontiguousarray(inputs["beta"], dtype=np.float32).reshape(1, D)

    Xb = np.ascontiguousarray(X).astype(bf)
    Xb8 = np.ascontiguousarray(X).astype(ml_dtypes.float8_e4m3)
    W31 = np.concatenate([W3, W1 @ W2], axis=1)            # [D, 128]
    W31h, W31l = _hilo16(
        np.ascontiguousarray(W31.reshape(8, 128, 128).transpose(1, 0, 2))
    )
    Ub = {
        k: np.ascontiguousarray(inputs[k], dtype=np.float32).astype(bf)
        for k in ("U1_0", "U2_0", "U1_1", "U2_1")
    }

    in_maps = []
    for c in range(NCORES):
        lo, hi = c * NLOC, (c + 1) * NLOC
        Xloc = np.ascontiguousarray(X[lo:hi])
        XlocThi, XlocTlo = _hilo16(
            np.ascontiguousarray(Xloc.T.reshape(8, 128, NLOC).transpose(1, 0, 2))
        )
        # S + I prepacked: the diagonal ride-along makes Xf' = recipD*(S+I)@Xf
        # exact in one matmul stream (diag weight ~1/rowsum, f8 error there
        # is negligible)
        SpI = S[lo:hi].copy()
        SpI[np.arange(NLOC), lo + np.arange(NLOC)] += 1.0
        STb = np.ascontiguousarray(
            SpI.T.reshape(NJC, 128, NLOC).transpose(1, 0, 2)
        ).astype(ml_dtypes.float8_e4m3)
        m = {
            "Xb": Xb,
            "Xb8": Xb8,
            "Xloc": Xloc,
            "XlocThi": XlocThi,
            "XlocTlo": XlocTlo,
            "STb": STb,
            "W31h": W31h,
            "W31l": W31l,
            "gamma": gamma,
            "beta": beta,
        }
        m.update(Ub)
        in_maps.append(m)
    return in_maps


def kernel(**inputs):
    from concourse.bass_utils import run_bass_kernel_spmd

    nc = _get_nc(_ln_identity(inputs))
    in_maps = make_in_maps(inputs)
    res = run_bass_kernel_spmd(nc, in_maps, core_ids=list(range(NCORES)))
    out = np.concatenate([res.results[c]["out"] for c in range(NCORES)], axis=0)
    return np.ascontiguousarray(out, dtype=np.float32)



# revision 76
# speedup vs baseline: 1.1188x; 1.0805x over previous
"""Trainium2 Bass kernel for nn_AdaptiveFunctionBlock (gnn_message_passing).

Row-shards N=4096 across 8 NeuronCores (512 rows each).  Host prepacks
bf16 / pre-transposed views of the inputs (X, X^T, S_loc^T, [W3|W1@W2],
U_s) so the device does no layout shuffles or casts of the raw inputs.
S^T and A^T live in SBUF for the whole kernel (no HBM scratch).

Per core:
  prologue: rowsums of S (PE ones-matmul), KP^T = [W3|W1W2]^T @ X^T
    (full, so no K/P AllGather), K1P1 row-major via PE transposes.
  step s: L = P@K^T (bf16 matmuls, f32 psum), exact per-row top-p via
    segment top-8 candidates -> sorted top-T -> f32 cumsum threshold
    (reference-faithful tie handling on step 2), softmax -> A (bf16),
    A^T via PE transposes into resident SBUF, then row-parallel GEMMs
    Xa' = A@Xa, Xf' = recipD*(S@Xf + Xf_diag), Z += Xf'@U1_s + Xa'@U2_s,
    with an AllGather of (Xf'|Xa') between steps and a small AllGather
    of K2P2 for the step-2 logits.  Finally LayerNorm(X+Z)*gamma+beta.
"""

import numpy as np

N = 4096
D = 1024
DA = 64
NCORES = 8
NLOC = N // NCORES          # 512
NT = NLOC // 128            # 4 row-tiles per core
NJC = N // 128              # 32 column chunks
P_TOP = 0.9
LN_EPS = 1e-5
T1, T2 = 16, 96             # top-p extraction depth per step
NSEG = 32
SEGW = N // NSEG            # 128
NEGINF = -3.0e38
POSINF = 3.0e38

_CACHE = {}


def _build(ln_id=False):
    import concourse.bass as bass
    import concourse.mybir as mybir
    from concourse import bacc, tile

    dt = mybir.dt
    f32 = dt.float32
    f32r = dt.float32r
    bf16 = dt.bfloat16
    f8 = dt.float8e4
    DR = mybir.MatmulPerfMode.DoubleRow
    Act = mybir.ActivationFunctionType
    Alu = mybir.AluOpType
    PSUM = bass.MemorySpace.PSUM
    DRAM = bass.MemorySpace.DRAM

    nc = bacc.Bacc(num_devices=NCORES)

    # ---------------- I/O ----------------
    f16 = dt.float16
    Xb_d = nc.dram_tensor("Xb", [N, D], bf16, kind="ExternalInput")
    Xb8_d = nc.dram_tensor("Xb8", [N, D], f8, kind="ExternalInput")
    Xloc_d = nc.dram_tensor("Xloc", [NLOC, D], f32, kind="ExternalInput")
    XlocThi_d = nc.dram_tensor("XlocThi", [128, 8, NLOC], f16, kind="ExternalInput")
    XlocTlo_d = nc.dram_tensor("XlocTlo", [128, 8, NLOC], f16, kind="ExternalInput")
    STb_d = nc.dram_tensor("STb", [128, NJC, NLOC], f8, kind="ExternalInput")
    W31h_d = nc.dram_tensor("W31h", [128, 8, 128], f16, kind="ExternalInput")
    W31l_d = nc.dram_tensor("W31l", [128, 8, 128], f16, kind="ExternalInput")
    U_d = [
        nc.dram_tensor("U1_0", [D, D], bf16, kind="ExternalInput"),
        nc.dram_tensor("U2_0", [D, D], bf16, kind="ExternalInput"),
        nc.dram_tensor("U1_1", [D, D], bf16, kind="ExternalInput"),
        nc.dram_tensor("U2_1", [D, D], bf16, kind="ExternalInput"),
    ]
    gamma_d = nc.dram_tensor("gamma", [1, D], f32, kind="ExternalInput")
    beta_d = nc.dram_tensor("beta", [1, D], f32, kind="ExternalInput")
    out_d = nc.dram_tensor("out", [NLOC, D], f32, kind="ExternalOutput")

    # ---------------- inline constants ----------------
    identb_c = nc.inline_tensor(
        np.eye(128, dtype=np.float32), name="identb_c"
    )
    ones1_c = nc.inline_tensor(np.ones((1, 128), dtype=np.float32), name="ones1_c")
    onesp_c = nc.inline_tensor(np.ones((128, 1), dtype=np.float32), name="onesp_c")

    rg = [list(range(NCORES))]
    Tmax = max(T1, T2)

    with tile.TileContext(nc) as tc:
        with (
            tc.tile_pool(name="const", bufs=1) as cpool,
            tc.tile_pool(name="res", bufs=1) as rpool,
            tc.tile_pool(name="big", bufs=1) as bpool,
            tc.tile_pool(name="stream", bufs=1) as stpool,
            tc.tile_pool(name="small", bufs=1) as spool,
            tc.tile_pool(name="psum", bufs=1, space=PSUM) as ppool,
            tc.tile_pool(name="dram", bufs=1, space=DRAM) as dpool,
        ):
            def ps_tile(name):
                return ppool.tile([128, 512], f32, tag="pb", bufs=8, name=name)

            def ps_tile_b(name):
                return ppool.tile([128, 512], bf16, tag="pb", bufs=8, name=name)

            # ---------- SBUF consts ----------
            identf_s = cpool.tile([128, 128], f32, name="identf_s")
            nc.sync.dma_start(identf_s[:], identb_c[:])
            identr_s = cpool.tile([128, 128], f32r, name="identr_s")
            nc.sync.dma_start(identr_s[:], identb_c[:].bitcast(f32r))
            identb_s = cpool.tile([128, 128], bf16, name="identb_s")
            nc.vector.tensor_copy(identb_s[:], identf_s[:])
            ones1_s = cpool.tile([1, 128], f32, name="ones1_s")
            nc.sync.dma_start(ones1_s[:], ones1_c[:])
            onespf_s = cpool.tile([128, 1], f32, name="onespf_s")
            nc.sync.dma_start(onespf_s[:], onesp_c[:])
            onespb_s = cpool.tile([128, 1], f8, name="onespb_s")
            nc.vector.tensor_copy(onespb_s[:], onespf_s[:])
            zerosT_s = cpool.tile([128, Tmax], f32, name="zerosT_s")
            nc.vector.memset(zerosT_s[:], 0.0)

            # ---------- collective DRAM buffers ----------
            # merged AllGather payload: Xf' (NLOC x D f8 viewed as f32) then
            # kp2 (64 x NLOC f32 flattened into 128 rows of 256)
            ag_m_in = dpool.tile([NLOC + 128, 256], f32, name="ag_m_in")
            ag_m_out = dpool.tile(
                [NCORES, NLOC + 128, 256], f32, addr_space="Shared", name="ag_m_out"
            )
            # prologue KP^T AllGather: each core contributes its local cols
            ag_kp1_in = dpool.tile([128, NLOC], f32, name="ag_kp1_in")
            ag_kp1_out = dpool.tile(
                [NCORES, 128, NLOC], f32, addr_space="Shared", name="ag_kp1_out"
            )
            ag_xa_in = dpool.tile([NLOC, D], bf16, name="ag_xa_in")
            ag_xa_out = dpool.tile(
                [NCORES, NLOC, D], bf16, addr_space="Shared", name="ag_xa_out"
            )


            # ---------- residents ----------
            recipD_s = rpool.tile([128, NT], f32, name="recipD_s")
            Z_s = rpool.tile([128, NT, D], f32, name="Z_s")
            ST_s = rpool.tile([128, NJC, NLOC], f8, name="ST_s")
            ATb_s = rpool.tile([128, NT, N], bf16, name="ATb_s")
            K1P1rm_s = rpool.tile([128, NJC * 128], bf16, tag="K1P1", bufs=1, name="K1P1rm_s")
            Xf_loc = rpool.tile([128, NT, D], bf16, name="Xf_loc")
            Xa_loc = rpool.tile([128, NT, D], bf16, name="Xa_loc")
            KP2_s = rpool.tile([128, NLOC], f32r, name="KP2_s")

            # ================= PROLOGUE =================
            # S^T resident (prepacked on host, fully contiguous load)
            nc.sync.dma_start(ST_s[:], STb_d[:])
            W31h_s = stpool.tile([128, 8, 128], f16, tag="x4k", bufs=2, name="W31h_s")
            nc.sync.dma_start(W31h_s[:], W31h_d[:])
            W31l_s = stpool.tile([128, 8, 128], f16, tag="x4k", bufs=2, name="W31l_s")
            nc.sync.dma_start(W31l_s[:], W31l_d[:])

            # local KP^T = W31^T @ Xloc^T  (fp16 hi/lo 3-pass: ~exact), then
            # AllGather the 512-col slices so every core gets full KP^T.
            xlh = bpool.tile([128, 8, NLOC], f16, tag="L", bufs=2, name="xlh")
            nc.sync.dma_start(xlh[:], XlocThi_d[:])
            xll = bpool.tile([128, 8, NLOC], f16, tag="L", bufs=2, name="xll")
            nc.sync.dma_start(xll[:], XlocTlo_d[:])
            kqlp = ps_tile("kqlp")
            for dc in range(8):
                nc.tensor.matmul(
                    kqlp[:], W31h_s[:, dc, :], xlh[:, dc, :],
                    start=(dc == 0), stop=False,
                )
                nc.tensor.matmul(
                    kqlp[:], W31l_s[:, dc, :], xlh[:, dc, :], start=False, stop=False
                )
                nc.tensor.matmul(
                    kqlp[:], W31h_s[:, dc, :], xll[:, dc, :],
                    start=False, stop=(dc == 7),
                )
            KPloc_s = spool.tile([128, NLOC], f32, tag="PT", bufs=2, name="KPloc_s")
            nc.scalar.activation(KPloc_s[:], kqlp[:], Act.Copy)
            PT1_s = spool.tile([128, NLOC], f32, tag="PT", bufs=2, name="PT1_s")
            nc.sync.dma_start(PT1_s[0:64, :], KPloc_s[64:128, :])
            nc.sync.dma_start(PT1_s[64:128, :], KPloc_s[64:128, :])
            nc.sync.dma_start(ag_kp1_in[:], KPloc_s[:])
            nc.gpsimd.collective_compute(
                "AllGather",
                Alu.bypass,
                replica_groups=rg,
                ins=[ag_kp1_in[:].opt()],
                outs=[ag_kp1_out[:].opt()],
            )

            # assemble K1 two-half layout + full KP^T from the gather
            KPT_s = bpool.tile([128, N], f32, tag="E", bufs=1, name="KPT_s")
            K1Th_s = rpool.tile([128, N // 2], f32, tag="KTh", bufs=1, name="K1Th_s")
            for co in range(8):
                nc.sync.dma_start(
                    KPT_s[:, co * 512 : (co + 1) * 512], ag_kp1_out[co, :, :]
                )
                ph = 0 if co < 4 else 64
                off = co if co < 4 else co - 4
                nc.sync.dma_start(
                    K1Th_s[ph : ph + 64, off * 512 : (off + 1) * 512],
                    ag_kp1_out[co, 0:64, :],
                )

            # K1P1 row-major bf16 (for K2P2 accumulation) via f32 PE transposes
            for jq in range(8):
                tpk = ps_tile(f"tpk{jq}")
                for b in range(4):
                    jc = jq * 4 + b
                    nc.tensor.transpose(
                        tpk[:, b * 128 : (b + 1) * 128],
                        KPT_s[:, jc * 128 : (jc + 1) * 128],
                        identf_s[:],
                    )
                nc.scalar.activation(
                    K1P1rm_s[:, jq * 512 : (jq + 1) * 512], tpk[:], Act.Copy
                )

            kp2ps = None
            K2Th_s = None
            PT2_s = None

            # ================= STEPS =================
            for s in range(2):
                T = T1 if s == 0 else T2
                if s == 0:
                    PTl, KTh = PT1_s, K1Th_s
                else:
                    PTl, KTh = PT2_s, K2Th_s
                Ab_t = [None] * NT

                def emit_post(it, s=s):
                    # A^T into resident SBUF (bf16 PE transposes), then the
                    # step-0 K2P2 accumulation for this row tile.
                    Ab = Ab_t[it]
                    for jq in range(8):
                        tpa = ps_tile_b(f"tpa{s}{it}{jq}")
                        for b in range(4):
                            jc = jq * 4 + b
                            nc.tensor.transpose(
                                tpa[:, b * 128 : (b + 1) * 128],
                                Ab[:, jc * 128 : (jc + 1) * 128],
                                identb_s[:],
                            )
                        nc.scalar.activation(
                            ATb_s[:, it, jq * 512 : (jq + 1) * 512], tpa[:], Act.Copy
                        )
                    if s == 0:
                        for jc in range(NJC):
                            nc.tensor.matmul(
                                kp2ps[:, it * 128 : (it + 1) * 128],
                                K1P1rm_s[:, jc * 128 : (jc + 1) * 128],
                                ATb_s[:, it, jc * 128 : (jc + 1) * 128],
                                start=(jc == 0),
                                stop=(jc == NJC - 1),
                            )

                # xf-part psum banks: 4 held (one dh half at a time)
                xfp = [[None] * 2 for _ in range(NT)]

                def emit_tsum(dh, s=s):
                    # Xf' = recipD * psum (S+I prepacked, diag included);
                    # frees the dh psum banks
                    for it in range(NT):
                        nc.vector.tensor_scalar_mul(
                            Xf_loc[:, it, dh * 512 : (dh + 1) * 512],
                            xfp[it][dh][:],
                            recipD_s[:, it : it + 1],
                        )

                def emit_xf_chunks(qlo, qhi, dh, s=s):
                    # DoubleRow fp8: two 128-row k-tiles per matmul
                    for q2 in range(qlo, qhi):
                        xch = stpool.tile(
                            [128, 2, 512], f8, tag="xch5", bufs=3, name="xch"
                        )
                        for kt in range(2):
                            jc = 2 * q2 + kt
                            if s == 0:
                                nc.sync.dma_start(
                                    xch[:, kt, :],
                                    Xb8_d[
                                        jc * 128 : (jc + 1) * 128,
                                        dh * 512 : (dh + 1) * 512,
                                    ],
                                )
                            else:
                                r_, blk_ = jc // NT, jc % NT
                                nc.sync.dma_start(
                                    xch[:, kt, :],
                                    ag_m_out[
                                        r_,
                                        blk_ * 128 : (blk_ + 1) * 128,
                                        dh * 128 : (dh + 1) * 128,
                                    ].bitcast(f8),
                                )
                        for itt in range(NT):
                            nc.tensor.matmul(
                                xfp[itt][dh][:],
                                ST_s[:, 2 * q2 : 2 * q2 + 2, itt * 128 : (itt + 1) * 128],
                                xch[:],
                                start=(q2 == 0),
                                stop=(q2 == NJC // 2 - 1),
                                perf_mode=DR,
                            )

                def emit_xa_all(s=s):
                    # one full-width stream, 8 psum banks
                    xap = [[ps_tile(f"xap{s}{it}{dh}") for dh in range(2)]
                           for it in range(NT)]
                    for jc in range(NJC):
                        xch = stpool.tile(
                            [128, D], bf16, tag="xchw", bufs=2, name="xcha"
                        )
                        if s == 0:
                            nc.sync.dma_start(
                                xch[:], Xb_d[jc * 128 : (jc + 1) * 128, :]
                            )
                        else:
                            r_, blk_ = jc // NT, jc % NT
                            nc.sync.dma_start(
                                xch[:],
                                ag_xa_out[r_, blk_ * 128 : (blk_ + 1) * 128, :],
                            )
                        for it in range(NT):
                            for dh in range(2):
                                nc.tensor.matmul(
                                    xap[it][dh][:],
                                    ATb_s[:, it, jc * 128 : (jc + 1) * 128],
                                    xch[:, dh * 512 : (dh + 1) * 512],
                                    start=(jc == 0),
                                    stop=(jc == NJC - 1),
                                )
                    for it in range(NT):
                        for dh in range(2):
                            nc.scalar.activation(
                                Xa_loc[:, it, dh * 512 : (dh + 1) * 512],
                                xap[it][dh][:],
                                Act.Copy,
                            )

                def emit_tpt(s=s):
                    # Xf^T / Xa^T (all tiles)
                    for src, dst in ((Xf_loc, XfT_s), (Xa_loc, XaT_s)):
                        for dc in range(8):
                            tpt = ps_tile_b(f"tpt{s}{dc}")
                            for it in range(NT):
                                nc.tensor.transpose(
                                    tpt[:, it * 128 : (it + 1) * 128],
                                    src[:, it, dc * 128 : (dc + 1) * 128],
                                    identb_s[:],
                                )
                            nc.scalar.activation(dst[:, dc, :], tpt[:], Act.Copy)

                def emit_u_all(s=s):
                    # Z = Xf'@U1_s + Xa'@U2_s (+ X on step 0): 8 psum banks
                    zp = [[ps_tile(f"zp{s}{it}{dh}") for dh in range(2)]
                          for it in range(NT)]
                    for u, XT in ((0, XfT_s), (1, XaT_s)):
                        for dc in range(8):
                            ubf = stpool.tile(
                                [128, D], bf16, tag="x2k", bufs=2, name="ubf"
                            )
                            nc.sync.dma_start(
                                ubf[:], U_d[2 * s + u][dc * 128 : (dc + 1) * 128, :]
                            )
                            for it in range(NT):
                                for dh in range(2):
                                    nc.tensor.matmul(
                                        zp[it][dh][:],
                                        XT[:, dc, it * 128 : (it + 1) * 128],
                                        ubf[:, dh * 512 : (dh + 1) * 512],
                                        start=(u == 0 and dc == 0),
                                        stop=(u == 1 and dc == 7),
                                    )
                    for it in range(NT):
                        for dh in range(2):
                            if s == 0:
                                xlf = stpool.tile(
                                    [128, 512], f32, tag="xth", bufs=2, name="xlf0"
                                )
                                nc.sync.dma_start(
                                    xlf[:],
                                    Xloc_d[
                                        it * 128 : (it + 1) * 128,
                                        dh * 512 : (dh + 1) * 512,
                                    ],
                                )
                                nc.vector.tensor_tensor(
                                    Z_s[:, it, dh * 512 : (dh + 1) * 512],
                                    zp[it][dh][:],
                                    xlf[:],
                                    Alu.add,
                                )
                            else:
                                nc.vector.tensor_tensor(
                                    Z_s[:, it, dh * 512 : (dh + 1) * 512],
                                    Z_s[:, it, dh * 512 : (dh + 1) * 512],
                                    zp[it][dh][:],
                                    Alu.add,
                                )

                def emit_lp(it, s=s):
                    L = bpool.tile([128, N], f32, tag="L", bufs=2, name="L")
                    for jc8 in range(8):
                        lp = ps_tile(f"lp{it}{jc8}")
                        kph = 0 if jc8 < 4 else 64
                        kpo = jc8 if jc8 < 4 else jc8 - 4
                        nc.tensor.matmul(
                            lp[:],
                            PTl[kph : kph + 64, it * 128 : (it + 1) * 128],
                            KTh[kph : kph + 64, kpo * 512 : (kpo + 1) * 512],
                            start=True,
                            stop=True,
                        )
                        nc.scalar.activation(
                            L[:, jc8 * 512 : (jc8 + 1) * 512], lp[:], Act.Copy
                        )
                    return L

                if s == 0:
                    # rowsums of S+I -> recipD
                    kp2ps = ps_tile("kp2ps")
                    for rit in range(NT):
                        rsp = ps_tile(f"rsp{rit}")
                        for jc in range(NJC):
                            nc.tensor.matmul(
                                rsp[:, 0:1],
                                ST_s[:, jc, rit * 128 : (rit + 1) * 128],
                                onespb_s[:],
                                start=(jc == 0),
                                stop=(jc == NJC - 1),
                            )
                        nc.vector.reciprocal(
                            recipD_s[:, rit : rit + 1], rsp[:, 0:1]
                        )

                L_t = [None] * NT

                def emit_topp(it, s=s, T=T):
                    L = L_t[it]
                    # --- candidates: per-segment top-8. Step 0 (k_i <= 11)
                    # captures all top-k_i with 8 wide segments; step 1 needs 32.
                    nseg = 8 if s == 0 else NSEG
                    segw = N // nseg
                    candw = nseg * 8
                    cand = spool.tile([128, 256], f32, tag="cand", bufs=2, name="cand")
                    for sg in range(nseg):
                        nc.vector.max(
                            cand[:, sg * 8 : sg * 8 + 8],
                            L[:, sg * segw : (sg + 1) * segw],
                        )
                    if s == 1:
                        cand_copy = spool.tile(
                            [128, 256], f32, tag="cand", bufs=2, name="cand_copy"
                        )
                        nc.vector.tensor_copy(cand_copy[:], cand[:])

                    # --- extract sorted top-T (destroys cand)
                    V = spool.tile([128, Tmax], f32, tag="V", bufs=2, name="V")
                    for rnd in range(T // 8):
                        nc.vector.max(V[:, rnd * 8 : rnd * 8 + 8], cand[:, 0:candw])
                        if rnd < T // 8 - 1:
                            nc.vector.match_replace(
                                cand[:, 0:candw],
                                V[:, rnd * 8 : rnd * 8 + 8],
                                cand[:, 0:candw],
                                NEGINF,
                            )

                    negm = spool.tile([128, 1], f32, tag="negm", bufs=2, name="negm")
                    nc.vector.tensor_scalar_mul(negm[:], V[:, 0:1], -1.0)

                    E = bpool.tile([128, N], f32, tag="E", bufs=1, name="E")
                    Zrow = spool.tile([128, 1], f32, tag="Zrow", bufs=2, name="Zrow")
                    nc.scalar.activation(
                        E[:], L[:], Act.Exp, bias=negm[:], accum_out=Zrow[:]
                    )
                    EV = spool.tile([128, Tmax], f32, tag="EV", bufs=2, name="EV")
                    nc.scalar.activation(EV[:, 0:T], V[:, 0:T], Act.Exp, bias=negm[:])
                    cs = spool.tile([128, Tmax], f32, tag="cs", bufs=2, name="cs")
                    nc.vector.tensor_tensor_scan(
                        cs[:, 0:T], EV[:, 0:T], zerosT_s[:, 0:T], 0.0, Alu.add, Alu.add
                    )
                    thr = spool.tile([128, 1], f32, tag="thr", bufs=2, name="thr")
                    nc.vector.tensor_scalar_mul(thr[:], Zrow[:], P_TOP)
                    kept = spool.tile([128, Tmax], f32, tag="kept", bufs=2, name="kept")
                    nc.vector.scalar_tensor_tensor(
                        kept[:, 0:T], cs[:, 0:T], thr[:], EV[:, 0:T],
                        Alu.subtract, Alu.is_lt,
                    )
                    scr1 = spool.tile([128, Tmax], f32, tag="scr1", bufs=1, name="scr1")
                    Drow = spool.tile([128, 1], f32, tag="Drow", bufs=2, name="Drow")
                    nc.vector.tensor_tensor(
                        scr1[:, 0:T], EV[:, 0:T], kept[:, 0:T], Alu.mult
                    )
                    nc.vector.tensor_reduce(
                        Drow[:], scr1[:, 0:T], mybir.AxisListType.X, Alu.add
                    )
                    nki = spool.tile([128, Tmax], f32, tag="nki", bufs=1, name="nki")
                    nc.vector.tensor_scalar(
                        nki[:, 0:T], kept[:, 0:T], 0.5, POSINF, Alu.is_lt, Alu.mult
                    )
                    scr2 = spool.tile([128, Tmax], f32, tag="scr2", bufs=1, name="scr2")
                    tau = spool.tile([128, 1], f32, tag="tau", bufs=2, name="tau")
                    nc.vector.tensor_tensor(
                        scr2[:, 0:T], nki[:, 0:T], V[:, 0:T], Alu.add
                    )
                    nc.vector.tensor_reduce(
                        tau[:], scr2[:, 0:T], mybir.AxisListType.X, Alu.min
                    )
                    recD = spool.tile([128, 1], f32, tag="recD", bufs=2, name="recD")
                    nc.vector.reciprocal(recD[:], Drow[:])
                    etau = spool.tile([128, 1], f32, tag="etau", bufs=2, name="etau")
                    nc.scalar.activation(etau[:], tau[:], Act.Exp, bias=negm[:])

                    if s == 1:
                        scrT = spool.tile(
                            [128, Tmax], f32, tag="scrT", bufs=1, name="scrT"
                        )
                        rr = spool.tile([128, 1], f32, tag="rr", bufs=2, name="rr")
                        nc.vector.scalar_tensor_tensor(
                            scrT[:, 0:T], V[:, 0:T], tau[:], kept[:, 0:T],
                            Alu.is_equal, Alu.mult,
                        )
                        nc.vector.tensor_reduce(
                            rr[:], scrT[:, 0:T], mybir.AxisListType.X, Alu.add
                        )
                        scr256 = spool.tile(
                            [128, 256], f32, tag="scrT", bufs=1, name="scr256"
                        )
                        ceq = spool.tile([128, 1], f32, tag="ceq", bufs=2, name="ceq")
                        nc.vector.tensor_scalar(
                            scr256[:], cand_copy[:], tau[:], None, Alu.is_equal
                        )
                        nc.vector.tensor_reduce(
                            ceq[:], scr256[:], mybir.AxisListType.X, Alu.add
                        )
                        # w2 = (ceq - r)/ceq * etau * recD  (spread over all ties)
                        wv = spool.tile([128, 1], f32, tag="wv", bufs=2, name="wv")
                        nc.vector.tensor_tensor(wv[:], ceq[:], rr[:], Alu.subtract)
                        nc.vector.tensor_tensor(wv[:], wv[:], etau[:], Alu.mult)
                        nc.vector.tensor_tensor(wv[:], wv[:], recD[:], Alu.mult)
                        rceq = spool.tile([128, 1], f32, tag="rceq", bufs=2, name="rceq")
                        nc.vector.reciprocal(rceq[:], ceq[:])
                        nc.vector.tensor_tensor(wv[:], wv[:], rceq[:], Alu.mult)

                    # --- A materialization: E := (E >= etau) * E;  Ab = E*recD
                    # (step 2 fuses the tie-drop correction into the scale)
                    nc.vector.scalar_tensor_tensor(
                        E[:], E[:], etau[:], E[:], Alu.is_ge, Alu.mult
                    )
                    Ab = bpool.tile([128, N], bf16, tag="XfT", bufs=2, name="Ab")
                    if s == 1:
                        eqw = rpool.tile([128, N], bf16, tag="K1P1", bufs=1, name="eqw")
                        nc.vector.tensor_scalar(
                            eqw[:], L[:], tau[:], wv[:], Alu.is_equal, Alu.mult
                        )
                        nc.vector.scalar_tensor_tensor(
                            Ab[:], E[:], recD[:], eqw[:], Alu.mult, Alu.subtract
                        )
                    else:
                        nc.scalar.activation(Ab[:], E[:], Act.Copy, scale=recD[:])
                    Ab_t[it] = Ab

                # per-row-tile: logits -> top-p -> A, xf chunks interleaved
                for it in range(NT):
                    L_t[it] = emit_lp(it)
                    dh = it // 2
                    if it % 2 == 0:
                        if dh == 1:
                            emit_tsum(0)
                        for itt in range(NT):
                            xfp[itt][dh] = ps_tile(f"xfp{s}{itt}{dh}")
                    emit_xf_chunks((it % 2) * 8, (it % 2) * 8 + 8, dh)
                    emit_topp(it)
                    if it >= 1:
                        emit_post(it - 1)
                emit_post(NT - 1)
                emit_tsum(1)

                XfT_s = bpool.tile(
                    [128, 8, NLOC], bf16, tag="L", bufs=2, name=f"XfT{s}"
                )
                XaT_s = bpool.tile(
                    [128, 8, NLOC], bf16, tag="L", bufs=2, name=f"XaT{s}"
                )

                if s == 0:
                    # free the kp2 psum bank
                    nc.scalar.activation(KP2_s[:], kp2ps[:], Act.Copy)

                if s == 0:
                    # gather Xf' + kp2 in ONE AllGather: both only depend on
                    # the xf-part / step-0 A^T, so this launches before the
                    # xa matmuls and overlaps them.
                    for it in range(NT):
                        xf8 = stpool.tile([128, D], f8, tag="x2k", bufs=2, name="xf8")
                        nc.scalar.activation(xf8[:], Xf_loc[:, it, :], Act.Copy)
                        nc.sync.dma_start(
                            ag_m_in[it * 128 : (it + 1) * 128, :],
                            xf8[:].bitcast(f32),
                        )
                    nc.sync.dma_start(
                        ag_m_in[NLOC : NLOC + 128, :].bitcast(f32r),
                        KP2_s[0:64, :],
                    )
                    nc.gpsimd.collective_compute(
                        "AllGather",
                        Alu.bypass,
                        replica_groups=rg,
                        ins=[ag_m_in[:].opt()],
                        outs=[ag_m_out[:].opt()],
                    )

                # ---------- xa stream, then transposes + U products
                emit_xa_all()

                if s == 0:
                    # stage + launch the Xa AllGather
                    for it in range(NT):
                        nc.sync.dma_start(
                            ag_xa_in[it * 128 : (it + 1) * 128, :], Xa_loc[:, it, :]
                        )
                    nc.gpsimd.collective_compute(
                        "AllGather",
                        Alu.bypass,
                        replica_groups=rg,
                        ins=[ag_xa_in[:].opt()],
                        outs=[ag_xa_out[:].opt()],
                    )

                emit_tpt()
                emit_u_all()

                if s == 0:
                    # K2 two-half layout (waits on the merged AllGather) + local PT2
                    K2Th_s = rpool.tile(
                        [128, N // 2], f32r, tag="KTh", bufs=1, name="K2Th_s"
                    )
                    for r in range(NCORES):
                        ph = 0 if r < 4 else 64
                        off = r if r < 4 else r - 4
                        nc.sync.dma_start(
                            K2Th_s[ph : ph + 64, off * NLOC : (off + 1) * NLOC],
                            ag_m_out[r, NLOC : NLOC + 128, :].bitcast(f32r),
                        )
                    PT2_s = spool.tile(
                        [128, NLOC], f32r, tag="PT", bufs=2, name="PT2_s"
                    )
                    nc.sync.dma_start(PT2_s[0:64, :], KP2_s[64:128, :])
                    nc.sync.dma_start(PT2_s[64:128, :], KP2_s[64:128, :])

            # ================= LayerNorm epilogue =================
            if not ln_id:
                gamma_s1 = rpool.tile([1, D], f32, tag="K1P1", bufs=1, name="gamma_s1")
                beta_s1 = rpool.tile([1, D], f32, tag="KTh", bufs=1, name="beta_s1")
                nc.sync.dma_start(gamma_s1[:], gamma_d[:])
                nc.sync.dma_start(beta_s1[:], beta_d[:])
                gamma_bc = bpool.tile([128, D], f32, tag="XfT", bufs=2, name="gamma_bc")
                beta_bc = bpool.tile([128, D], f32, tag="XfT", bufs=2, name="beta_bc")
            for dh in range(2 if not ln_id else 0):
                gps = ps_tile(f"gps{dh}")
                nc.tensor.matmul(
                    gps[:],
                    ones1_s[:],
                    gamma_s1[:, dh * 512 : (dh + 1) * 512],
                    start=True,
                    stop=True,
                )
                nc.scalar.activation(
                    gamma_bc[:, dh * 512 : (dh + 1) * 512], gps[:], Act.Copy
                )
                bps = ps_tile(f"bps{dh}")
                nc.tensor.matmul(
                    bps[:],
                    ones1_s[:],
                    beta_s1[:, dh * 512 : (dh + 1) * 512],
                    start=True,
                    stop=True,
                )
                nc.scalar.activation(
                    beta_bc[:, dh * 512 : (dh + 1) * 512], bps[:], Act.Copy
                )

            inv_d = 1.0 / D
            for it in range(NT):
                ve = nc.vector
                Y = Z_s[:, it, :]          # Z_s already holds Z + X in f32
                sY = spool.tile([128, 1], f32, tag="sY", bufs=2, name="sY")
                nc.vector.tensor_reduce(sY[:], Y, mybir.AxisListType.X, Alu.add)
                scrB = bpool.tile([128, D], f32, tag="L", bufs=2, name="scrB")
                sY2 = spool.tile([128, 1], f32, tag="sY2", bufs=2, name="sY2")
                nc.scalar.activation(scrB[:], Y, Act.Square, accum_out=sY2[:])
                mu = spool.tile([128, 1], f32, tag="mu", bufs=2, name="mu")
                nc.vector.tensor_scalar_mul(mu[:], sY[:], inv_d)
                ex2 = spool.tile([128, 1], f32, tag="ex2", bufs=2, name="ex2")
                nc.vector.tensor_scalar_mul(ex2[:], sY2[:], inv_d)
                musq = spool.tile([128, 1], f32, tag="musq", bufs=2, name="musq")
                nc.vector.tensor_tensor(musq[:], mu[:], mu[:], Alu.mult)
                var = spool.tile([128, 1], f32, tag="var", bufs=2, name="var")
                nc.vector.tensor_tensor(var[:], ex2[:], musq[:], Alu.subtract)
                vpe = spool.tile([128, 1], f32, tag="vpe", bufs=2, name="vpe")
                nc.vector.tensor_scalar_add(vpe[:], var[:], LN_EPS)
                sd = spool.tile([128, 1], f32, tag="sd", bufs=2, name="sd")
                nc.scalar.activation(sd[:], vpe[:], Act.Sqrt)
                rstd = spool.tile([128, 1], f32, tag="rstd", bufs=2, name="rstd")
                nc.vector.reciprocal(rstd[:], sd[:])
                nmr = spool.tile([128, 1], f32, tag="nmr", bufs=2, name="nmr")
                nc.vector.tensor_tensor(nmr[:], mu[:], rstd[:], Alu.mult)
                nc.vector.tensor_scalar_mul(nmr[:], nmr[:], -1.0)
                tnorm = bpool.tile([128, D], f32, tag="L", bufs=2, name="tnorm")
                nc.scalar.activation(
                    tnorm[:], Y, Act.Identity, bias=nmr[:], scale=rstd[:]
                )
                if ln_id:
                    yout = tnorm
                else:
                    ve.tensor_tensor(tnorm[:], tnorm[:], gamma_bc[:], Alu.mult)
                    yout = bpool.tile([128, D], f32, tag="L", bufs=2, name="yout")
                    ve.tensor_tensor(yout[:], tnorm[:], beta_bc[:], Alu.add)
                nc.sync.dma_start(out_d[it * 128 : (it + 1) * 128, :], yout[:])

    nc.finalize()
    return nc


def _get_nc(ln_id=False):
    key = ("nc", ln_id)
    if key not in _CACHE:
        _CACHE[key] = _build(ln_id)
    return _CACHE[key]


def _ln_identity(inputs):
    g = np.asarray(inputs["gamma"], dtype=np.float32).ravel()
    b = np.asarray(inputs["beta"], dtype=np.float32).ravel()
    return bool(np.all(g == 1.0) and np.all(b == 0.0))


def _hilo16(x):
    # fp16 hi/lo split: x ~= hi + lo with hi = fp16(x), lo = fp16(x - hi)
    hi = np.ascontiguousarray(x, dtype=np.float32).astype(np.float16)
    lo = (x - hi.astype(np.float32)).astype(np.float16)
    return np.ascontiguousarray(hi), np.ascontiguousarray(lo)


def make_in_maps(inputs):
    import ml_dtypes

    bf = ml_dtypes.bfloat16
    X = np.ascontiguousarray(inputs["X"], dtype=np.float32)
    S = np.ascontiguousarray(inputs["S"], dtype=np.float32)
    W1 = np.asarray(inputs["W1"], dtype=np.float32)
    W2 = np.asarray(inputs["W2"], dtype=np.float32)
    W3 = np.asarray(inputs["W3"], dtype=np.float32)
    gamma = np.ascontiguousarray(inputs["gamma"], dtype=np.float32).reshape(1, D)
    beta = np.ascontiguousarray(inputs["beta"], dtype=np.float32).reshape(1, D)

    Xb = np.ascontiguousarray(X).astype(bf)
    Xb8 = np.ascontiguousarray(X).astype(ml_dtypes.float8_e4m3)
    W31 = np.concatenate([W3, W1 @ W2], axis=1)            # [D, 128]
    W31h, W31l = _hilo16(
        np.ascontiguousarray(W31.reshape(8, 128, 128).transpose(1, 0, 2))
    )
    Ub = {
        k: np.ascontiguousarray(inputs[k], dtype=np.float32).astype(bf)
        for k in ("U1_0", "U2_0", "U1_1", "U2_1")
    }

    in_maps = []
    for c in range(NCORES):
        lo, hi = c * NLOC, (c + 1) * NLOC
        Xloc = np.ascontiguousarray(X[lo:hi])
        XlocThi, XlocTlo = _hilo16(
            np.ascontiguousarray(Xloc.T.reshape(8, 128, NLOC).transpose(1, 0, 2))
        )
        # S + I prepacked: the diagonal ride-along makes Xf' = recipD*(S+I)@Xf
        # exact in one matmul stream (diag weight ~1/rowsum, f8 error there
        # is negligible)
        SpI = S[lo:hi].copy()
        SpI[np.arange(NLOC), lo + np.arange(NLOC)] += 1.0
        STb = np.ascontiguousarray(
            SpI.T.reshape(NJC, 128, NLOC).transpose(1, 0, 2)
        ).astype(ml_dtypes.float8_e4m3)
        m = {
            "Xb": Xb,
            "Xb8": Xb8,
            "Xloc": Xloc,
            "XlocThi": XlocThi,
            "XlocTlo": XlocTlo,
            "STb": STb,
            "W31h": W31h,
            "W31l": W31l,
            "gamma": gamma,
            "beta": beta,
        }
        m.update(Ub)
        in_maps.append(m)
    return in_maps


def kernel(**inputs):
    from concourse.bass_utils import run_bass_kernel_spmd

    nc = _get_nc(_ln_identity(inputs))
    in_maps = make_in_maps(inputs)
    res = run_bass_kernel_spmd(nc, in_maps, core_ids=list(range(NCORES)))
    out = np.concatenate([res.results[c]["out"] for c in range(NCORES)], axis=0)
    return np.ascontiguousarray(out, dtype=np.float32)

